# revision 13
# baseline (speedup 1.0000x reference)
"""Trainium2 Bass kernel for a dense transformer block.

Math (per batch element b of x[4, 2048, 768]):
    x = x + Attn(LN1(x));  x = x + MLP(LN2(x))   (12 heads, hidden 3072, exact gelu)

Sharding: 8 cores = (batch b in 0..3) x (sequence half g in 0..1). Each core
computes the full block for its own 1024 query tokens; k/v projections are
recomputed over the full 2048-token sequence of its batch element (no
collectives). Per-core token order is rotated so own tokens are always
columns 0..1023 -> one SPMD program for all cores.

Precision plan (validated off-line vs the fp32 reference, rel err ~9e-3
against a 2e-2 budget):
  - z1 (LN1 output), qkv/v weights: fp8 e4m3 -> DoubleRow matmuls (2x K rate).
    Weights are pre-scaled x16; the x16 on q and k folds into the softmax
    exp scale, the x16 on v cancels against a 16.0 "ones" column that also
    produces the softmax denominator.
  - softmax probabilities: fp8 e4m3, exp(s - 4.25) so the max stays ~110
    (fp8e4 max 240); the bias cancels in the normalization.
  - q/k activations, attn output, proj, LN2, fc1/fc2, h: bf16.
  - residual stream: bf16 in SBUF, f32 accumulation in PSUM, f32 output.

Attention structure per head pair (2hp, 2hp+1): score matmuls have K=64 so
the two heads run concurrently in the two row-halves of the PE array
(tile_position derives from base_partition). attn@v is a DoubleRow fp8
matmul with M=65 (64 v dims + denominator column). The denominator
reciprocal uses reciprocal_approx_fast and is broadcast across partitions
on the idle GPSIMD engine (no DRAM round-trip).

LayerNorm rstd is exp(-0.5*ln(var+eps)) so ACT only ever needs the
{exp,ln} table set plus gelu -- minimizes ACT_TABLE_LOAD switches.
"""

import sys

import numpy as np

sys.path.insert(0, "/opt/trn_rl_repo")

import ml_dtypes  # noqa: E402

import concourse.bacc as bacc  # noqa: E402
import concourse.mybir as mybir  # noqa: E402
import concourse.tile as tile  # noqa: E402
from concourse.bass_utils import run_bass_kernel_spmd  # noqa: E402

F32 = mybir.dt.float32
BF16 = mybir.dt.bfloat16
FP8 = mybir.dt.float8e4
AF = mybir.ActivationFunctionType
OP = mybir.AluOpType
DR = mybir.MatmulPerfMode.DoubleRow

P = 128
D = 768
DC = D // P            # 6 chunks of the model dim
H = 12
HD = 64
HID = 3072
HC = HID // P          # 24 chunks of the mlp hidden dim
EPS = 1e-5

W8 = 16.0              # fp8 weight pre-scale for qkv/v
EXP_BIAS = -4.25       # exp(s + EXP_BIAS); cancels in softmax
EXP_SCALE = (HD ** -0.5) / (W8 * W8)   # descales the x16 on q and x16 on k

NB = 4                 # batch
NT = 2048              # tokens per batch element (keys)
NO = NT // 2           # own tokens per core (queries)
N_CORES = 8

NPBF16 = ml_dtypes.bfloat16
NPFP8 = ml_dtypes.float8_e4m3


def _build_nc(nt, no, with_qk_bias, with_fc2_bias, reps=1, upto=99):
    """Build + schedule the SPMD Bass program (one core's view)."""
    nc = bacc.Bacc("TRN2", target_bir_lowering=False, debug=False,
                   num_devices=N_CORES)

    xT = nc.dram_tensor("xT", [P, DC, nt], BF16, kind="ExternalInput")
    resid = nc.dram_tensor("resid", [P, DC, no], BF16, kind="ExternalInput")
    wqkT = nc.dram_tensor("wqkT", [P, DC, 2 * D], FP8, kind="ExternalInput")
    wvT = nc.dram_tensor("wvT", [P, DC, D], FP8, kind="ExternalInput")
    pwT = nc.dram_tensor("pwT", [P, DC, D], BF16, kind="ExternalInput")
    w1T = nc.dram_tensor("w1T", [6, P, DC, 512], BF16, kind="ExternalInput")
    w2T = nc.dram_tensor("w2T", [6, P, HC, P], BF16, kind="ExternalInput")
    qk_bias = nc.dram_tensor("qk_bias", [P, 2 * DC], F32, kind="ExternalInput")
    b1p = nc.dram_tensor("b1p", [P, HC], F32, kind="ExternalInput")
    fc2_b = nc.dram_tensor("fc2_b", [P, DC], F32, kind="ExternalInput")
    outT = nc.dram_tensor("outT", [P, DC, no], F32, kind="ExternalOutput")
    dbg = {}
    if upto < 99:
        dbg["z1"] = nc.dram_tensor("dbg_z1", [P, DC, nt], FP8,
                                   kind="ExternalOutput")
        dbg["k"] = nc.dram_tensor("dbg_k", [P, DC, nt], BF16,
                                  kind="ExternalOutput")
        dbg["q"] = nc.dram_tensor("dbg_q", [P, DC, no], BF16,
                                  kind="ExternalOutput")
        dbg["v"] = nc.dram_tensor("dbg_v", [P, nt // 256, 2 * H, 80], FP8,
                                  kind="ExternalOutput")
        dbg["attn"] = nc.dram_tensor("dbg_attn", [P, DC, 512], BF16,
                                     kind="ExternalOutput")
        dbg["pT"] = nc.dram_tensor("dbg_pT", [P, nt // P, 512], FP8,
                                   kind="ExternalOutput")
        dbg["x1"] = nc.dram_tensor("dbg_x1", [P, DC, 512], BF16,
                                   kind="ExternalOutput")
        dbg["po"] = nc.dram_tensor("dbg_po", [P, 512], F32,
                                   kind="ExternalOutput")
        dbg["rec"] = nc.dram_tensor("dbg_rec", [1, 512], F32,
                                    kind="ExternalOutput")
        dbg["rbc"] = nc.dram_tensor("dbg_rbc", [HD, 512], F32,
                                    kind="ExternalOutput")

    with tile.TileContext(nc) as tc:
        for _ in range(reps):
            _emit(tc, nc, dict(
                xT=xT, resid=resid, wqkT=wqkT, wvT=wvT, pwT=pwT, w1T=w1T,
                w2T=w2T, qk_bias=qk_bias, b1p=b1p, fc2_b=fc2_b, outT=outT,
            ), nt, no, with_qk_bias, with_fc2_bias, upto, dbg)

    nc.compile()
    return nc


def _ln_stats(nc, ps_pool, tmps, ones_bf, eps_sb, zero_sb, x_src, sl_src,
              nmb, rstdb, sl_dst):
    """Per-token -mean (bf16) and rstd (bf16) over the model dim for the
    512-token slice `sl_src` of channel-major bf16 x_src [P, DC, n], written
    to slice `sl_dst` of nmb/rstdb.
    rstd = exp(-0.5*ln(var+eps)) keeps ACT on the {exp,ln} table set."""
    dc = DC
    # sum(x) over the model dim via ones-matmul (partition reduction)
    ps_s = ps_pool.tile([P, 512], F32, tag="mm")
    for c in range(dc):
        nc.tensor.matmul(ps_s, ones_bf, x_src[:, c, sl_src],
                         start=(c == 0), stop=(c == dc - 1))
    nc.vector.tensor_scalar_mul(nmb[:, sl_dst], ps_s, -1.0 / D)
    # sum(x^2): square on DVE into recycled bf16 tiles, then ones-matmul
    ps_q = ps_pool.tile([P, 512], F32, tag="mm")
    for c in range(dc):
        xsq = tmps.tile([P, 512], BF16, tag="xsq", bufs=3)
        nc.vector.tensor_mul(xsq, x_src[:, c, sl_src], x_src[:, c, sl_src])
        nc.tensor.matmul(ps_q, ones_bf, xsq,
                         start=(c == 0), stop=(c == dc - 1))
    var = tmps.tile([P, 512], F32, tag="var", bufs=1)
    nc.vector.tensor_scalar_mul(var, ps_q, 1.0 / D)
    msq = tmps.tile([P, 512], F32, tag="msq", bufs=1)
    nc.vector.tensor_mul(msq, nmb[:, sl_dst], nmb[:, sl_dst])
    nc.vector.tensor_tensor(var, var, msq, OP.subtract)
    lnv = tmps.tile([P, 512], F32, tag="lnv", bufs=1)
    nc.scalar.activation(lnv, var, AF.Ln, bias=eps_sb[:, 0:1])
    nc.scalar.activation(rstdb[:, sl_dst], lnv, AF.Exp, bias=zero_sb[:, 0:1],
                         scale=-0.5)


def _emit(tc, nc, io, nt, no, with_qk_bias, with_fc2_bias, upto=99, dbg=None):
    dc, hc = DC, HC
    ngk = nt // 512        # 512-wide groups over all tokens
    ngq = no // 512        # 512-wide groups over own tokens (pipeline stages)
    mt_n = nt // P         # 128-wide key tiles
    mtp_n = mt_n // 2      # pairs of key tiles (DoubleRow K chunks)
    _stack = []

    def _pool(*a, **k):
        p = tc.alloc_tile_pool(*a, **k)
        _stack.append(p)
        return p

    def _rel(p):
        assert _stack[-1] is p
        _stack.pop()
        p.release()

    def _cut():
        for p in reversed(_stack):
            p.release()
        _stack.clear()

    # ---- long-lived pools (created first, released last) -------------------
    consts = _pool(name="consts", bufs=1)
    tmps = _pool(name="tmps", bufs=2)
    ps_mm = _pool(name="ps_mm", bufs=2, space="PSUM")
    ps_sc = _pool(name="ps_sc", bufs=2, space="PSUM")
    ps_po = _pool(name="ps_po", bufs=2, space="PSUM")
    p_kT = _pool(name="p_kT", bufs=1)
    p_qT = _pool(name="p_qT", bufs=1)
    p_v = _pool(name="p_v", bufs=1)
    p_rec = _pool(name="p_rec", bufs=2)

    ones_bf = consts.tile([P, P], BF16)
    nc.vector.memset(ones_bf, 1.0)
    eps_sb = consts.tile([P, 1], F32)
    nc.vector.memset(eps_sb, EPS)
    zero_sb = consts.tile([P, 1], F32)
    nc.vector.memset(zero_sb, 0.0)
    expb_sb = consts.tile([P, 1], F32)
    nc.vector.memset(expb_sb, EXP_BIAS)
    qkb_sb = consts.tile([P, 2 * dc], F32)
    nc.sync.dma_start(qkb_sb, io["qk_bias"][:, :])
    b1p_sb = consts.tile([P, hc], F32)
    nc.sync.dma_start(b1p_sb, io["b1p"][:, :])
    fc2b_sb = consts.tile([P, dc], F32)
    nc.sync.dma_start(fc2b_sb, io["fc2_b"][:, :])

    kT = p_kT.tile([P, dc, nt], BF16)
    qT = p_qT.tile([P, dc, no], BF16)
    # v: [P, key-tile-pair, 2*head + tile-in-pair, 80]; col 64 = 16.0 for the
    # softmax denominator, cols 65..79 pad the fp8 DoubleRow stride to 80.
    v_pair = p_v.tile([P, mtp_n, 2 * H, 80], FP8)

    # ---- phase-1/2 pools (released before attention pools are created) -----
    p_wqk = _pool(name="p_wqk", bufs=1)
    p_z1 = _pool(name="p_z1", bufs=1)
    p_st1 = _pool(name="p_st1", bufs=1)
    p_xT = _pool(name="p_xT", bufs=1)

    wqk_sb = p_wqk.tile([P, dc, 2 * D], FP8, tag="wqk")
    nc.sync.dma_start(wqk_sb, io["wqkT"][:, :, :])
    wv_sb = p_wqk.tile([P, dc, D], FP8, tag="wv")
    nc.sync.dma_start(wv_sb, io["wvT"][:, :, :])

    xT_sb = p_xT.tile([P, dc, nt], BF16)
    for c in range(dc):
        for hh in range(nt // 1024):
            hsl = slice(hh * 1024, (hh + 1) * 1024)
            nc.sync.dma_start(xT_sb[:, c, hsl], io["xT"][:, c, hsl])

    nmb = p_st1.tile([P, nt], BF16, tag="nmb")
    rstdb = p_st1.tile([P, nt], BF16, tag="rstdb")
    z1T = p_z1.tile([P, dc, nt], FP8)

    # ---------------- Phase 1: LN1 stats + z1 (fp8) -------------------------
    for ng in range(ngk):
        sl = slice(ng * 512, (ng + 1) * 512)
        _ln_stats(nc, ps_mm, tmps, ones_bf, eps_sb, zero_sb, xT_sb, sl, nmb,
                  rstdb, sl)
        for c in range(dc):
            t = tmps.tile([P, 512], BF16, tag="lnt", bufs=3)
            nc.vector.tensor_add(t, xT_sb[:, c, sl], nmb[:, sl])
            nc.vector.tensor_mul(z1T[:, c, sl], t, rstdb[:, sl])
    _rel(p_xT)
    _rel(p_st1)
    if upto <= 1:
        nc.sync.dma_start(dbg["z1"][:, :, :], z1T)
        _cut()
        return

    # ---------------- Phase 2: qkv projections (fp8 DoubleRow) --------------
    # k and q chunk-by-chunk so attention on head pair hp can start as soon
    # as chunk hp is done.
    for cc in range(dc):
        for ng in range(ngk):
            sl = slice(ng * 512, (ng + 1) * 512)
            ps = ps_mm.tile([P, 512], F32, tag="mm")
            for c in range(3):
                nc.tensor.matmul(
                    ps, wqk_sb[:, 2 * c:2 * c + 2,
                               D + cc * P:D + (cc + 1) * P],
                    z1T[:, 2 * c:2 * c + 2, sl],
                    start=(c == 0), stop=(c == 2), perf_mode=DR)
            if with_qk_bias:
                nc.vector.tensor_scalar(kT[:, cc, sl], ps,
                                        qkb_sb[:, dc + cc:dc + cc + 1], None,
                                        OP.add)
            else:
                nc.vector.tensor_copy(kT[:, cc, sl], ps)
        for g in range(ngq):
            sl = slice(g * 512, (g + 1) * 512)
            ps = ps_mm.tile([P, 512], F32, tag="mm")
            for c in range(3):
                nc.tensor.matmul(
                    ps, wqk_sb[:, 2 * c:2 * c + 2, cc * P:(cc + 1) * P],
                    z1T[:, 2 * c:2 * c + 2, sl],
                    start=(c == 0), stop=(c == 2), perf_mode=DR)
            if with_qk_bias:
                nc.vector.tensor_scalar(qT[:, cc, sl], ps,
                                        qkb_sb[:, cc:cc + 1], None, OP.add)
            else:
                nc.vector.tensor_copy(qT[:, cc, sl], ps)

    # v (token-major): half 0 first so heads 0..5 complete early.
    nc.vector.memset(v_pair[:, :, :, 64:65], W8)
    for half in range(2):
        rhs_sl = slice(half * 384, (half + 1) * 384)
        for mt in range(mt_n):
            ps = ps_mm.tile([P, 384], F32, tag="mm")
            for c in range(3):
                nc.tensor.matmul(
                    ps, z1T[:, 2 * c:2 * c + 2, mt * P:(mt + 1) * P],
                    wv_sb[:, 2 * c:2 * c + 2, rhs_sl],
                    start=(c == 0), stop=(c == 2), perf_mode=DR)
            dst = v_pair[:, mt // 2,
                         slice(12 * half + (mt % 2), 12 * half + 12, 2), 0:64]
            nc.vector.tensor_copy(
                dst, ps.rearrange("p (h d) -> p h d", d=64))

    if upto <= 2:
        nc.sync.dma_start(dbg["z1"][:, :, :], z1T)
        nc.sync.dma_start(dbg["k"][:, :, :], kT)
        nc.sync.dma_start(dbg["q"][:, :, :], qT)
        nc.sync.dma_start(dbg["v"][:, :, :, :], v_pair)
        _rel(p_z1)
        _rel(p_wqk)
        _cut()
        return
    _rel(p_z1)
    _rel(p_wqk)

    # ---- attention/MLP pools (reuse the phase-1/2 SBUF) --------------------
    p_pT = _pool(name="p_pT", bufs=3)
    p_attn = _pool(name="p_attn", bufs=2)
    p_x1 = _pool(name="p_x1", bufs=1)
    p_z2 = _pool(name="p_z2", bufs=1)
    p_h = _pool(name="p_h", bufs=1)
    p_w12 = _pool(name="p_w12", bufs=2)
    p_pw = _pool(name="p_pw", bufs=1)
    p_res = _pool(name="p_res", bufs=2)
    p_st2 = _pool(name="p_st2", bufs=2)
    p_x2 = _pool(name="p_x2", bufs=2)

    pw_sb = p_pw.tile([P, dc, D], BF16)
    nc.sync.dma_start(pw_sb, io["pwT"][:, :, :])
    x1T = p_x1.tile([P, dc, no], BF16)

    def attention(g):
        sl = slice(g * 512, (g + 1) * 512)
        attnT = p_attn.tile([P, dc, 512], BF16, tag="attnT",
                            name=f"attnT_{g}")
        for hp in range(dc):
            pTs = [p_pT.tile([P, mt_n, 512], FP8, tag="pT",
                             name=f"pT_{g}_{2 * hp + s}") for s in range(2)]

            for mtp in range(mtp_n):
                scs = [ps_sc.tile([P, 2, 512], F32, tag="sc",
                                  name=f"sc_{g}_{hp}_{mtp}_{s}")
                       for s in range(2)]
                for s2 in range(2):
                    mt = 2 * mtp + s2
                    msl = slice(mt * P, (mt + 1) * P)
                    # two heads run concurrently in the two row-halves
                    nc.tensor.matmul(scs[0][:, s2], kT[0:HD, hp, msl],
                                     qT[0:HD, hp, sl])
                    nc.tensor.matmul(scs[1][:, s2], kT[HD:P, hp, msl],
                                     qT[HD:P, hp, sl])
                for s in range(2):
                    nc.scalar.activation(pTs[s][:, 2 * mtp:2 * mtp + 2],
                                         scs[s][:, :, :], AF.Exp,
                                         bias=expb_sb[:, 0:1],
                                         scale=EXP_SCALE)
            if dbg and upto <= 3 and g == 0 and hp == 0:
                nc.sync.dma_start(dbg["pT"][:, :, :], pTs[0])
            for s in range(2):
                h = 2 * hp + s
                po = ps_po.tile([P, 512], F32, tag="po")
                for mtp in range(mtp_n):
                    nc.tensor.matmul(po[0:65],
                                     v_pair[:, mtp, 2 * h:2 * h + 2, 0:65],
                                     pTs[s][:, 2 * mtp:2 * mtp + 2, :],
                                     start=(mtp == 0), stop=(mtp == mtp_n - 1),
                                     perf_mode=DR)
                den = p_rec.tile([1, 512], F32, tag="den", bufs=2)
                nc.vector.tensor_copy(den, po[64:65, :])
                rec = p_rec.tile([1, 512], F32, tag="rec", bufs=2)
                nc.vector.reciprocal_approx_fast(out=rec, in_=den)
                rbc = p_rec.tile([HD, 512], F32, tag="rbc", bufs=2)
                nc.gpsimd.partition_broadcast(rbc, rec, channels=HD)
                if dbg and upto <= 3 and g == 0 and h == 0:
                    po_d = tmps.tile([P, 512], F32, tag="po_d", bufs=1)
                    nc.vector.tensor_copy(po_d, po)
                    nc.sync.dma_start(dbg["po"][:, :], po_d)
                    nc.sync.dma_start(dbg["rec"][:, :], rec)
                    nc.sync.dma_start(dbg["rbc"][:, :], rbc)
                nc.vector.tensor_tensor(attnT[s * HD:(s + 1) * HD, hp, :],
                                        po[0:HD, :], rbc, OP.mult)
        return attnT

    def proj_ln2(g, attnT):
        sl = slice(g * 512, (g + 1) * 512)
        res_sb = p_res.tile([P, dc, 512], BF16, tag="res", name=f"res_{g}")
        for c in range(dc):
            nc.sync.dma_start(res_sb[:, c], io["resid"][:, c, sl])
        for ec in range(dc):
            ps = ps_mm.tile([P, 512], F32, tag="mm")
            for c in range(dc):
                nc.tensor.matmul(ps, pw_sb[:, c, ec * P:(ec + 1) * P],
                                 attnT[:, c, :], start=(c == 0),
                                 stop=(c == dc - 1))
            nc.vector.tensor_add(x1T[:, ec, sl], ps, res_sb[:, ec])
        # LN2 on this 512-token group
        nm2 = p_st2.tile([P, 512], BF16, tag="nm2", name=f"nm2_{g}")
        rstd2 = p_st2.tile([P, 512], BF16, tag="rstd2", name=f"rstd2_{g}")
        _ln_stats(nc, ps_mm, tmps, ones_bf, eps_sb, zero_sb, x1T, sl, nm2,
                  rstd2, slice(0, 512))
        z2T = p_z2.tile([P, dc, 512], BF16, tag="z2", name=f"z2_{g}")
        for c in range(dc):
            t = tmps.tile([P, 512], BF16, tag="lnt", bufs=3)
            nc.vector.tensor_add(t, x1T[:, c, sl], nm2)
            nc.vector.tensor_mul(z2T[:, c], t, rstd2)
        return z2T

    def mlp(g, z2T):
        sl = slice(g * 512, (g + 1) * 512)
        hT = p_h.tile([P, hc, 512], BF16, tag="hT", name=f"hT_{g}")
        for i in range(6):
            w1s = p_w12.tile([P, dc, 512], BF16, tag="w1", name=f"w1_{g}_{i}")
            nc.sync.dma_start(w1s, io["w1T"][i])
            for ci in range(4):
                cc = i * 4 + ci
                ps = ps_mm.tile([P, 512], F32, tag="mm")
                for c in range(dc):
                    nc.tensor.matmul(ps, w1s[:, c, ci * P:(ci + 1) * P],
                                     z2T[:, c], start=(c == 0),
                                     stop=(c == dc - 1))
                nc.scalar.activation(hT[:, cc], ps, AF.Gelu,
                                     bias=b1p_sb[:, cc:cc + 1])
        for ec in range(dc):
            w2s = p_w12.tile([P, hc, P], BF16, tag="w2", name=f"w2_{g}_{ec}")
            nc.sync.dma_start(w2s, io["w2T"][ec])
            ps = ps_mm.tile([P, 512], F32, tag="mm")
            for c in range(hc):
                nc.tensor.matmul(ps, w2s[:, c], hT[:, c],
                                 start=(c == 0), stop=(c == hc - 1))
            x2 = p_x2.tile([P, 512], F32, tag="x2")
            if with_fc2_bias:
                t = tmps.tile([P, 512], F32, tag="f2t")
                nc.vector.tensor_scalar(t, ps, fc2b_sb[:, ec:ec + 1], None,
                                        OP.add)
                nc.vector.tensor_add(x2, t, x1T[:, ec, sl])
            else:
                nc.vector.tensor_add(x2, ps, x1T[:, ec, sl])
            nc.sync.dma_start(io["outT"][:, ec, sl], x2)

    # pipeline: attn(g1) is emitted before mlp(g0) so the exp-bound attention
    # window overlaps the MLP matmuls on the tensor engine.
    attn0 = attention(0)
    if upto <= 3:
        nc.sync.dma_start(dbg["attn"][:, :, :], attn0)
        _cut()
        return
    z2_0 = proj_ln2(0, attn0)
    if upto <= 4:
        nc.sync.dma_start(dbg["x1"][:, :, :], x1T[:, :, 0:512])
        nc.sync.dma_start(dbg["attn"][:, :, :], attn0)
        _cut()
        return
    attn1 = attention(1)
    mlp(0, z2_0)
    z2_1 = proj_ln2(1, attn1)
    mlp(1, z2_1)
    _cut()


# --------------------------------------------------------------------------
# Host side
# --------------------------------------------------------------------------

_NC_CACHE = {}


def _get_nc(nt, no, with_qk_bias, with_fc2_bias, reps=1, upto=99):
    key = (nt, no, with_qk_bias, with_fc2_bias, reps, upto)
    if key not in _NC_CACHE:
        _NC_CACHE[key] = _build_nc(nt, no, with_qk_bias, with_fc2_bias, reps,
                                   upto)
    return _NC_CACHE[key]


def _prep_weights(ln1_w, ln1_b, qkv_w, qkv_b, proj_w, proj_b,
                  ln2_w, ln2_b, fc1_w, fc1_b, fc2_w, fc2_b):
    w_qkv = qkv_w * ln1_w[None, :]
    b_qkv = qkv_w @ ln1_b + qkv_b
    pb = proj_b + proj_w @ b_qkv[2 * D:]
    w1 = fc1_w * ln2_w[None, :]
    b1p = fc1_b + fc1_w @ ln2_b

    def col(v, chunks):
        return np.ascontiguousarray(v.reshape(chunks, P).T.astype(np.float32))

    def sb(wT, chunks, npdt):
        # [K, M] -> [P, chunks, M] with K = chunks*P (SBUF layout)
        k, m = wT.shape
        return np.ascontiguousarray(
            wT.reshape(chunks, P, m).transpose(1, 0, 2).astype(npdt))

    w1_s = sb(w1.T, DC, NPBF16)                   # [P, DC, 3072]
    w2_s = sb(fc2_w.T, HC, NPBF16)                # [P, HC, 768]
    shared = {
        "wqkT": sb(w_qkv[:2 * D].T * W8, DC, NPFP8),
        "wvT": sb(w_qkv[2 * D:].T * W8, DC, NPFP8),
        "pwT": sb(proj_w.T, DC, NPBF16),
        "w1T": np.ascontiguousarray(
            w1_s.reshape(P, DC, 6, 512).transpose(2, 0, 1, 3)),
        "w2T": np.ascontiguousarray(
            w2_s.reshape(P, HC, 6, P).transpose(2, 0, 1, 3)),
        "qk_bias": col(b_qkv[:2 * D] * W8, 2 * DC),
        "b1p": col(b1p, HC),
        "fc2_b": col(fc2_b, DC),
    }
    flags = (bool(np.any(b_qkv[:2 * D])), bool(np.any(fc2_b)))
    return shared, pb, flags


def run_on_device(inputs, trace=False):
    x = np.asarray(inputs["x"], dtype=np.float32)
    nb, nt, d = x.shape
    no = nt // 2
    args = {k: np.asarray(v, dtype=np.float32) for k, v in inputs.items()
            if k != "x"}
    shared, pb, (f_qk, f_f2) = _prep_weights(
        args["ln1_w"], args["ln1_b"], args["qkv_w"], args["qkv_b"],
        args["proj_w"], args["proj_b"], args["ln2_w"], args["ln2_b"],
        args["fc1_w"], args["fc1_b"], args["fc2_w"], args["fc2_b"])

    nc = _get_nc(nt, no, f_qk, f_f2)

    in_maps = []
    for core in range(N_CORES):
        b, g = divmod(core, 2)
        xr = np.roll(x[b], -g * no, axis=0)
        m = dict(shared)
        m["xT"] = np.ascontiguousarray(
            xr.T.reshape(DC, P, nt).transpose(1, 0, 2)).astype(NPBF16)
        rs = x[b, g * no:(g + 1) * no].T + pb[:, None]
        m["resid"] = np.ascontiguousarray(
            rs.reshape(DC, P, no).transpose(1, 0, 2)).astype(NPBF16)
        in_maps.append(m)

    res = run_bass_kernel_spmd(nc, in_maps, core_ids=list(range(N_CORES)),
                               trace=trace)
    out = np.empty((nb, nt, d), dtype=np.float32)
    for core in range(N_CORES):
        b, g = divmod(core, 2)
        o = res.results[core]["outT"]          # [P, DC, no]
        out[b, g * no:(g + 1) * no, :] = o.transpose(1, 0, 2).reshape(d, no).T
    return out, res


def kernel(**inputs) -> np.ndarray:
    out, _ = run_on_device(inputs, trace=False)
    return out


# revision 14
# speedup vs baseline: 1.0199x; 1.0199x over previous
"""Trainium2 Bass kernel for a dense transformer block.

Math (per batch element b of x[4, 2048, 768]):
    x = x + Attn(LN1(x));  x = x + MLP(LN2(x))   (12 heads, hidden 3072, exact gelu)

Sharding: 8 cores = (batch b in 0..3) x (sequence half g in 0..1). Each core
computes the full block for its own 1024 query tokens; k/v projections are
recomputed over the full 2048-token sequence of its batch element (no
collectives). Per-core token order is rotated so own tokens are always
columns 0..1023 -> one SPMD program for all cores.

Precision plan (validated off-line vs the fp32 reference, rel err ~9e-3
against a 2e-2 budget):
  - z1 (LN1 output), qkv/v weights: fp8 e4m3 -> DoubleRow matmuls (2x K rate).
    Weights are pre-scaled x16; the x16 on q and k folds into the softmax
    exp scale, the x16 on v cancels against a 16.0 "ones" column that also
    produces the softmax denominator.
  - softmax probabilities: fp8 e4m3, exp(s - 4.25) so the max stays ~110
    (fp8e4 max 240); the bias cancels in the normalization.
  - q/k activations, attn output, proj, LN2, fc1/fc2, h: bf16.
  - residual stream: bf16 in SBUF, f32 accumulation in PSUM, f32 output.

Attention structure per head pair (2hp, 2hp+1): score matmuls have K=64 so
the two heads run concurrently in the two row-halves of the PE array
(tile_position derives from base_partition). attn@v is a DoubleRow fp8
matmul with M=65 (64 v dims + denominator column). The denominator
reciprocal uses reciprocal_approx_fast and is broadcast across partitions
on the idle GPSIMD engine (no DRAM round-trip).

LayerNorm rstd is exp(-0.5*ln(var+eps)) so ACT only ever needs the
{exp,ln} table set plus gelu -- minimizes ACT_TABLE_LOAD switches.
"""

import sys

import numpy as np

sys.path.insert(0, "/opt/trn_rl_repo")

import ml_dtypes  # noqa: E402

import concourse.bacc as bacc  # noqa: E402
import concourse.mybir as mybir  # noqa: E402
import concourse.tile as tile  # noqa: E402
from concourse.bass_utils import run_bass_kernel_spmd  # noqa: E402

F32 = mybir.dt.float32
BF16 = mybir.dt.bfloat16
FP8 = mybir.dt.float8e4
AF = mybir.ActivationFunctionType
OP = mybir.AluOpType
DR = mybir.MatmulPerfMode.DoubleRow

P = 128
D = 768
DC = D // P            # 6 chunks of the model dim
H = 12
HD = 64
HID = 3072
HC = HID // P          # 24 chunks of the mlp hidden dim
EPS = 1e-5

W8 = 16.0              # fp8 weight pre-scale for qkv/v
EXP_BIAS = -4.25       # exp(s + EXP_BIAS); cancels in softmax
EXP_SCALE = (HD ** -0.5) / (W8 * W8)   # descales the x16 on q and x16 on k

NB = 4                 # batch
NT = 2048              # tokens per batch element (keys)
NO = NT // 2           # own tokens per core (queries)
N_CORES = 8

NPBF16 = ml_dtypes.bfloat16
NPFP8 = ml_dtypes.float8_e4m3


def _build_nc(nt, no, with_qk_bias, with_fc2_bias, reps=1, upto=99):
    """Build + schedule the SPMD Bass program (one core's view)."""
    nc = bacc.Bacc("TRN2", target_bir_lowering=False, debug=False,
                   num_devices=N_CORES)

    xT = nc.dram_tensor("xT", [P, DC, nt], BF16, kind="ExternalInput")
    resid = nc.dram_tensor("resid", [P, DC, no], BF16, kind="ExternalInput")
    wqkT = nc.dram_tensor("wqkT", [P, DC, 2 * D], FP8, kind="ExternalInput")
    wvT = nc.dram_tensor("wvT", [P, DC, D], FP8, kind="ExternalInput")
    pwT = nc.dram_tensor("pwT", [P, DC, D], BF16, kind="ExternalInput")
    w1T = nc.dram_tensor("w1T", [6, P, DC, 512], BF16, kind="ExternalInput")
    w2T = nc.dram_tensor("w2T", [6, P, HC, P], BF16, kind="ExternalInput")
    qk_bias = nc.dram_tensor("qk_bias", [P, 2 * DC], F32, kind="ExternalInput")
    b1p = nc.dram_tensor("b1p", [P, HC], F32, kind="ExternalInput")
    fc2_b = nc.dram_tensor("fc2_b", [P, DC], F32, kind="ExternalInput")
    outT = nc.dram_tensor("outT", [P, DC, no], F32, kind="ExternalOutput")
    dbg = {}
    if upto < 99:
        dbg["z1"] = nc.dram_tensor("dbg_z1", [P, DC, nt], FP8,
                                   kind="ExternalOutput")
        dbg["k"] = nc.dram_tensor("dbg_k", [P, DC, nt], BF16,
                                  kind="ExternalOutput")
        dbg["q"] = nc.dram_tensor("dbg_q", [P, DC, no], BF16,
                                  kind="ExternalOutput")
        dbg["v"] = nc.dram_tensor("dbg_v", [P, nt // 256, 2 * H, 80], FP8,
                                  kind="ExternalOutput")
        dbg["attn"] = nc.dram_tensor("dbg_attn", [P, DC, 512], BF16,
                                     kind="ExternalOutput")
        dbg["pT"] = nc.dram_tensor("dbg_pT", [P, nt // P, 512], FP8,
                                   kind="ExternalOutput")
        dbg["x1"] = nc.dram_tensor("dbg_x1", [P, DC, 512], BF16,
                                   kind="ExternalOutput")
        dbg["po"] = nc.dram_tensor("dbg_po", [P, 512], F32,
                                   kind="ExternalOutput")
        dbg["rec"] = nc.dram_tensor("dbg_rec", [1, 512], F32,
                                    kind="ExternalOutput")
        dbg["rbc"] = nc.dram_tensor("dbg_rbc", [HD, 512], F32,
                                    kind="ExternalOutput")

    with tile.TileContext(nc) as tc:
        for _ in range(reps):
            _emit(tc, nc, dict(
                xT=xT, resid=resid, wqkT=wqkT, wvT=wvT, pwT=pwT, w1T=w1T,
                w2T=w2T, qk_bias=qk_bias, b1p=b1p, fc2_b=fc2_b, outT=outT,
            ), nt, no, with_qk_bias, with_fc2_bias, upto, dbg)

    nc.compile()
    return nc


def _ln_stats(nc, ps_pool, tmps, ones_bf, eps_sb, zero_sb, x_src, sl_src,
              nmb, rstdb, sl_dst):
    """Per-token -mean (bf16) and rstd (bf16) over the model dim for the
    512-token slice `sl_src` of channel-major bf16 x_src [P, DC, n], written
    to slice `sl_dst` of nmb/rstdb.
    rstd = exp(-0.5*ln(var+eps)) keeps ACT on the {exp,ln} table set."""
    dc = DC
    # sum(x) over the model dim via ones-matmul (partition reduction)
    ps_s = ps_pool.tile([P, 512], F32, tag="mm")
    for c in range(dc):
        nc.tensor.matmul(ps_s, ones_bf, x_src[:, c, sl_src],
                         start=(c == 0), stop=(c == dc - 1))
    nc.vector.tensor_scalar_mul(nmb[:, sl_dst], ps_s, -1.0 / D)
    # sum(x^2): square on DVE into recycled bf16 tiles, then ones-matmul
    ps_q = ps_pool.tile([P, 512], F32, tag="mm")
    for c in range(dc):
        xsq = tmps.tile([P, 512], BF16, tag="xsq", bufs=3)
        nc.vector.tensor_mul(xsq, x_src[:, c, sl_src], x_src[:, c, sl_src])
        nc.tensor.matmul(ps_q, ones_bf, xsq,
                         start=(c == 0), stop=(c == dc - 1))
    var = tmps.tile([P, 512], F32, tag="var", bufs=1)
    nc.vector.tensor_scalar_mul(var, ps_q, 1.0 / D)
    msq = tmps.tile([P, 512], F32, tag="msq", bufs=1)
    nc.vector.tensor_mul(msq, nmb[:, sl_dst], nmb[:, sl_dst])
    nc.vector.tensor_tensor(var, var, msq, OP.subtract)
    lnv = tmps.tile([P, 512], F32, tag="lnv", bufs=1)
    nc.scalar.activation(lnv, var, AF.Ln, bias=eps_sb[:, 0:1])
    nc.scalar.activation(rstdb[:, sl_dst], lnv, AF.Exp, bias=zero_sb[:, 0:1],
                         scale=-0.5)


def _emit(tc, nc, io, nt, no, with_qk_bias, with_fc2_bias, upto=99, dbg=None):
    dc, hc = DC, HC
    ngk = nt // 512        # 512-wide groups over all tokens
    ngq = no // 512        # 512-wide groups over own tokens (pipeline stages)
    mt_n = nt // P         # 128-wide key tiles
    mtp_n = mt_n // 2      # pairs of key tiles (DoubleRow K chunks)
    _stack = []

    def _pool(*a, **k):
        p = tc.alloc_tile_pool(*a, **k)
        _stack.append(p)
        return p

    def _rel(p):
        assert _stack[-1] is p
        _stack.pop()
        p.release()

    def _cut():
        for p in reversed(_stack):
            p.release()
        _stack.clear()

    # ---- long-lived pools (created first, released last) -------------------
    consts = _pool(name="consts", bufs=1)
    tmps = _pool(name="tmps", bufs=2)
    ps_mm = _pool(name="ps_mm", bufs=2, space="PSUM")
    ps_sc = _pool(name="ps_sc", bufs=2, space="PSUM")
    ps_po = _pool(name="ps_po", bufs=2, space="PSUM")
    p_kT = _pool(name="p_kT", bufs=1)
    p_qT = _pool(name="p_qT", bufs=1)
    p_v = _pool(name="p_v", bufs=1)
    p_rec = _pool(name="p_rec", bufs=2)

    ones_bf = consts.tile([P, P], BF16)
    nc.vector.memset(ones_bf, 1.0)
    eps_sb = consts.tile([P, 1], F32)
    nc.vector.memset(eps_sb, EPS)
    zero_sb = consts.tile([P, 1], F32)
    nc.vector.memset(zero_sb, 0.0)
    expb_sb = consts.tile([P, 1], F32)
    nc.vector.memset(expb_sb, EXP_BIAS)
    qkb_sb = consts.tile([P, 2 * dc], F32)
    nc.sync.dma_start(qkb_sb, io["qk_bias"][:, :])
    b1p_sb = consts.tile([P, hc], F32)
    nc.sync.dma_start(b1p_sb, io["b1p"][:, :])
    fc2b_sb = consts.tile([P, dc], F32)
    nc.sync.dma_start(fc2b_sb, io["fc2_b"][:, :])

    kT = p_kT.tile([P, dc, nt], BF16)
    qT = p_qT.tile([P, dc, no], BF16)
    # v: [P, key-tile-pair, 2*head + tile-in-pair, 80]; col 64 = 16.0 for the
    # softmax denominator, cols 65..79 pad the fp8 DoubleRow stride to 80.
    v_pair = p_v.tile([P, mtp_n, 2 * H, 80], FP8)

    # ---- phase-1/2 pools (released before attention pools are created) -----
    p_wqk = _pool(name="p_wqk", bufs=1)
    p_z1 = _pool(name="p_z1", bufs=1)
    p_st1 = _pool(name="p_st1", bufs=1)
    p_xT = _pool(name="p_xT", bufs=1)

    wqk_sb = p_wqk.tile([P, dc, 2 * D], FP8, tag="wqk")
    nc.sync.dma_start(wqk_sb, io["wqkT"][:, :, :])
    wv_sb = p_wqk.tile([P, dc, D], FP8, tag="wv")
    nc.sync.dma_start(wv_sb, io["wvT"][:, :, :])

    xT_sb = p_xT.tile([P, dc, nt], BF16)
    for c in range(dc):
        for hh in range(nt // 1024):
            hsl = slice(hh * 1024, (hh + 1) * 1024)
            nc.sync.dma_start(xT_sb[:, c, hsl], io["xT"][:, c, hsl])

    nmb = p_st1.tile([P, nt], BF16, tag="nmb")
    rstdb = p_st1.tile([P, nt], BF16, tag="rstdb")
    z1T = p_z1.tile([P, dc, nt], FP8)

    # ---------------- Phase 1: LN1 stats + z1 (fp8) -------------------------
    for ng in range(ngk):
        sl = slice(ng * 512, (ng + 1) * 512)
        _ln_stats(nc, ps_mm, tmps, ones_bf, eps_sb, zero_sb, xT_sb, sl, nmb,
                  rstdb, sl)
        for c in range(dc):
            t = tmps.tile([P, 512], BF16, tag="lnt", bufs=3)
            nc.vector.tensor_add(t, xT_sb[:, c, sl], nmb[:, sl])
            nc.vector.tensor_mul(z1T[:, c, sl], t, rstdb[:, sl])
    _rel(p_xT)
    _rel(p_st1)
    if upto <= 1:
        nc.sync.dma_start(dbg["z1"][:, :, :], z1T)
        _cut()
        return

    # ---------------- Phase 2: qkv projections (fp8 DoubleRow) --------------
    # k and q chunk-by-chunk so attention on head pair hp can start as soon
    # as chunk hp is done.
    for cc in range(dc):
        for ng in range(ngk):
            sl = slice(ng * 512, (ng + 1) * 512)
            ps = ps_mm.tile([P, 512], F32, tag="mm")
            for c in range(3):
                nc.tensor.matmul(
                    ps, wqk_sb[:, 2 * c:2 * c + 2,
                               D + cc * P:D + (cc + 1) * P],
                    z1T[:, 2 * c:2 * c + 2, sl],
                    start=(c == 0), stop=(c == 2), perf_mode=DR)
            if with_qk_bias:
                nc.vector.tensor_scalar(kT[:, cc, sl], ps,
                                        qkb_sb[:, dc + cc:dc + cc + 1], None,
                                        OP.add)
            else:
                nc.vector.tensor_copy(kT[:, cc, sl], ps)
        for g in range(ngq):
            sl = slice(g * 512, (g + 1) * 512)
            ps = ps_mm.tile([P, 512], F32, tag="mm")
            for c in range(3):
                nc.tensor.matmul(
                    ps, wqk_sb[:, 2 * c:2 * c + 2, cc * P:(cc + 1) * P],
                    z1T[:, 2 * c:2 * c + 2, sl],
                    start=(c == 0), stop=(c == 2), perf_mode=DR)
            if with_qk_bias:
                nc.vector.tensor_scalar(qT[:, cc, sl], ps,
                                        qkb_sb[:, cc:cc + 1], None, OP.add)
            else:
                nc.vector.tensor_copy(qT[:, cc, sl], ps)

    # v (token-major): half 0 first so heads 0..5 complete early.
    nc.vector.memset(v_pair[:, :, :, 64:65], W8)
    for half in range(2):
        rhs_sl = slice(half * 384, (half + 1) * 384)
        for mt in range(mt_n):
            ps = ps_mm.tile([P, 384], F32, tag="mm")
            for c in range(3):
                nc.tensor.matmul(
                    ps, z1T[:, 2 * c:2 * c + 2, mt * P:(mt + 1) * P],
                    wv_sb[:, 2 * c:2 * c + 2, rhs_sl],
                    start=(c == 0), stop=(c == 2), perf_mode=DR)
            dst = v_pair[:, mt // 2,
                         slice(12 * half + (mt % 2), 12 * half + 12, 2), 0:64]
            nc.vector.tensor_copy(
                dst, ps.rearrange("p (h d) -> p h d", d=64))

    if upto <= 2:
        nc.sync.dma_start(dbg["z1"][:, :, :], z1T)
        nc.sync.dma_start(dbg["k"][:, :, :], kT)
        nc.sync.dma_start(dbg["q"][:, :, :], qT)
        nc.sync.dma_start(dbg["v"][:, :, :, :], v_pair)
        _rel(p_z1)
        _rel(p_wqk)
        _cut()
        return
    _rel(p_z1)
    _rel(p_wqk)

    # ---- attention/MLP pools (reuse the phase-1/2 SBUF) --------------------
    p_pT = _pool(name="p_pT", bufs=3)
    p_attn = _pool(name="p_attn", bufs=2)
    p_x1 = _pool(name="p_x1", bufs=1)
    p_z2 = _pool(name="p_z2", bufs=1)
    p_h = _pool(name="p_h", bufs=1)
    p_w12 = _pool(name="p_w12", bufs=2)
    p_pw = _pool(name="p_pw", bufs=1)
    p_res = _pool(name="p_res", bufs=2)
    p_st2 = _pool(name="p_st2", bufs=2)
    p_x2 = _pool(name="p_x2", bufs=2)

    pw_sb = p_pw.tile([P, dc, D], BF16)
    nc.sync.dma_start(pw_sb, io["pwT"][:, :, :])
    x1T = p_x1.tile([P, dc, no], BF16)

    def attention(g, attnT, pairs):
        sl = slice(g * 512, (g + 1) * 512)
        for hp in pairs:
            pTs = [p_pT.tile([P, mt_n, 512], FP8, tag="pT",
                             name=f"pT_{g}_{2 * hp + s}") for s in range(2)]
            for mtp in range(mtp_n):
                scs = [ps_sc.tile([P, 2, 512], F32, tag="sc",
                                  name=f"sc_{g}_{hp}_{mtp}_{s}")
                       for s in range(2)]
                for s2 in range(2):
                    mt = 2 * mtp + s2
                    msl = slice(mt * P, (mt + 1) * P)
                    # two heads run concurrently in the two row-halves
                    nc.tensor.matmul(scs[0][:, s2], kT[0:HD, hp, msl],
                                     qT[0:HD, hp, sl])
                    nc.tensor.matmul(scs[1][:, s2], kT[HD:P, hp, msl],
                                     qT[HD:P, hp, sl])
                for s in range(2):
                    nc.scalar.activation(pTs[s][:, 2 * mtp:2 * mtp + 2],
                                         scs[s][:, :, :], AF.Exp,
                                         bias=expb_sb[:, 0:1],
                                         scale=EXP_SCALE)
            if dbg and upto <= 3 and g == 0 and hp == 0:
                nc.sync.dma_start(dbg["pT"][:, :, :], pTs[0])
            for s in range(2):
                h = 2 * hp + s
                po = ps_po.tile([P, 512], F32, tag="po")
                for mtp in range(mtp_n):
                    nc.tensor.matmul(po[0:65],
                                     v_pair[:, mtp, 2 * h:2 * h + 2, 0:65],
                                     pTs[s][:, 2 * mtp:2 * mtp + 2, :],
                                     start=(mtp == 0), stop=(mtp == mtp_n - 1),
                                     perf_mode=DR)
                den = p_rec.tile([1, 512], F32, tag="den", bufs=2)
                nc.vector.tensor_copy(den, po[64:65, :])
                rec = p_rec.tile([1, 512], F32, tag="rec", bufs=2)
                nc.vector.reciprocal_approx_fast(out=rec, in_=den)
                rbc = p_rec.tile([HD, 512], F32, tag="rbc", bufs=2)
                nc.gpsimd.partition_broadcast(rbc, rec, channels=HD)
                if dbg and upto <= 3 and g == 0 and h == 0:
                    po_d = tmps.tile([P, 512], F32, tag="po_d", bufs=1)
                    nc.vector.tensor_copy(po_d, po)
                    nc.sync.dma_start(dbg["po"][:, :], po_d)
                    nc.sync.dma_start(dbg["rec"][:, :], rec)
                    nc.sync.dma_start(dbg["rbc"][:, :], rbc)
                nc.vector.tensor_tensor(attnT[s * HD:(s + 1) * HD, hp, :],
                                        po[0:HD, :], rbc, OP.mult)

    def proj_ln2(g, attnT):
        sl = slice(g * 512, (g + 1) * 512)
        res_sb = p_res.tile([P, dc, 512], BF16, tag="res", name=f"res_{g}")
        for c in range(dc):
            nc.sync.dma_start(res_sb[:, c], io["resid"][:, c, sl])
        for ec in range(dc):
            ps = ps_mm.tile([P, 512], F32, tag="mm")
            for c in range(dc):
                nc.tensor.matmul(ps, pw_sb[:, c, ec * P:(ec + 1) * P],
                                 attnT[:, c, :], start=(c == 0),
                                 stop=(c == dc - 1))
            nc.vector.tensor_add(x1T[:, ec, sl], ps, res_sb[:, ec])
        # LN2 on this 512-token group
        nm2 = p_st2.tile([P, 512], BF16, tag="nm2", name=f"nm2_{g}")
        rstd2 = p_st2.tile([P, 512], BF16, tag="rstd2", name=f"rstd2_{g}")
        _ln_stats(nc, ps_mm, tmps, ones_bf, eps_sb, zero_sb, x1T, sl, nm2,
                  rstd2, slice(0, 512))
        z2T = p_z2.tile([P, dc, 512], BF16, tag="z2", name=f"z2_{g}")
        for c in range(dc):
            t = tmps.tile([P, 512], BF16, tag="lnt", bufs=3)
            nc.vector.tensor_add(t, x1T[:, c, sl], nm2)
            nc.vector.tensor_mul(z2T[:, c], t, rstd2)
        return z2T

    def fc1_slices(g, z2T, hT, slices):
        for i in slices:
            w1s = p_w12.tile([P, dc, 512], BF16, tag="w1", name=f"w1_{g}_{i}")
            nc.sync.dma_start(w1s, io["w1T"][i])
            for ci in range(4):
                cc = i * 4 + ci
                ps = ps_mm.tile([P, 512], F32, tag="mm")
                for c in range(dc):
                    nc.tensor.matmul(ps, w1s[:, c, ci * P:(ci + 1) * P],
                                     z2T[:, c], start=(c == 0),
                                     stop=(c == dc - 1))
                nc.scalar.activation(hT[:, cc], ps, AF.Gelu,
                                     bias=b1p_sb[:, cc:cc + 1])

    def fc2(g, hT):
        sl = slice(g * 512, (g + 1) * 512)
        for ec in range(dc):
            w2s = p_w12.tile([P, hc, P], BF16, tag="w2", name=f"w2_{g}_{ec}")
            nc.sync.dma_start(w2s, io["w2T"][ec])
            ps = ps_mm.tile([P, 512], F32, tag="mm")
            for c in range(hc):
                nc.tensor.matmul(ps, w2s[:, c], hT[:, c],
                                 start=(c == 0), stop=(c == hc - 1))
            x2 = p_x2.tile([P, 512], F32, tag="x2")
            if with_fc2_bias:
                t = tmps.tile([P, 512], F32, tag="f2t")
                nc.vector.tensor_scalar(t, ps, fc2b_sb[:, ec:ec + 1], None,
                                        OP.add)
                nc.vector.tensor_add(x2, t, x1T[:, ec, sl])
            else:
                nc.vector.tensor_add(x2, ps, x1T[:, ec, sl])
            nc.sync.dma_start(io["outT"][:, ec, sl], x2)

    # pipeline: attention(g1) emission is interleaved with fc1(g0) in
    # chunks so the exp-bound attention window keeps the tensor engine fed
    # (and the HAM clock-gate warm) with MLP matmuls; fc2(g0) follows to
    # fill the tail of the exp window.
    attnT0 = p_attn.tile([P, dc, 512], BF16, tag="attnT", name="attnT_0")
    attention(0, attnT0, range(dc))
    if upto <= 3:
        nc.sync.dma_start(dbg["attn"][:, :, :], attnT0)
        _cut()
        return
    z2_0 = proj_ln2(0, attnT0)
    if upto <= 4:
        nc.sync.dma_start(dbg["x1"][:, :, :], x1T[:, :, 0:512])
        nc.sync.dma_start(dbg["attn"][:, :, :], attnT0)
        _cut()
        return
    attnT1 = p_attn.tile([P, dc, 512], BF16, tag="attnT", name="attnT_1")
    hT0 = p_h.tile([P, hc, 512], BF16, tag="hT", name="hT_0")
    for chunk in range(3):
        attention(1, attnT1, [2 * chunk, 2 * chunk + 1])
        fc1_slices(0, z2_0, hT0, [2 * chunk, 2 * chunk + 1])
    fc2(0, hT0)
    z2_1 = proj_ln2(1, attnT1)
    hT1 = p_h.tile([P, hc, 512], BF16, tag="hT", name="hT_1")
    fc1_slices(1, z2_1, hT1, range(6))
    fc2(1, hT1)
    _cut()


# --------------------------------------------------------------------------
# Host side
# --------------------------------------------------------------------------

_NC_CACHE = {}


def _get_nc(nt, no, with_qk_bias, with_fc2_bias, reps=1, upto=99):
    key = (nt, no, with_qk_bias, with_fc2_bias, reps, upto)
    if key not in _NC_CACHE:
        _NC_CACHE[key] = _build_nc(nt, no, with_qk_bias, with_fc2_bias, reps,
                                   upto)
    return _NC_CACHE[key]


def _prep_weights(ln1_w, ln1_b, qkv_w, qkv_b, proj_w, proj_b,
                  ln2_w, ln2_b, fc1_w, fc1_b, fc2_w, fc2_b):
    w_qkv = qkv_w * ln1_w[None, :]
    b_qkv = qkv_w @ ln1_b + qkv_b
    pb = proj_b + proj_w @ b_qkv[2 * D:]
    w1 = fc1_w * ln2_w[None, :]
    b1p = fc1_b + fc1_w @ ln2_b

    def col(v, chunks):
        return np.ascontiguousarray(v.reshape(chunks, P).T.astype(np.float32))

    def sb(wT, chunks, npdt):
        # [K, M] -> [P, chunks, M] with K = chunks*P (SBUF layout)
        k, m = wT.shape
        return np.ascontiguousarray(
            wT.reshape(chunks, P, m).transpose(1, 0, 2).astype(npdt))

    w1_s = sb(w1.T, DC, NPBF16)                   # [P, DC, 3072]
    w2_s = sb(fc2_w.T, HC, NPBF16)                # [P, HC, 768]
    shared = {
        "wqkT": sb(w_qkv[:2 * D].T * W8, DC, NPFP8),
        "wvT": sb(w_qkv[2 * D:].T * W8, DC, NPFP8),
        "pwT": sb(proj_w.T, DC, NPBF16),
        "w1T": np.ascontiguousarray(
            w1_s.reshape(P, DC, 6, 512).transpose(2, 0, 1, 3)),
        "w2T": np.ascontiguousarray(
            w2_s.reshape(P, HC, 6, P).transpose(2, 0, 1, 3)),
        "qk_bias": col(b_qkv[:2 * D] * W8, 2 * DC),
        "b1p": col(b1p, HC),
        "fc2_b": col(fc2_b, DC),
    }
    flags = (bool(np.any(b_qkv[:2 * D])), bool(np.any(fc2_b)))
    return shared, pb, flags


def run_on_device(inputs, trace=False):
    x = np.asarray(inputs["x"], dtype=np.float32)
    nb, nt, d = x.shape
    no = nt // 2
    args = {k: np.asarray(v, dtype=np.float32) for k, v in inputs.items()
            if k != "x"}
    shared, pb, (f_qk, f_f2) = _prep_weights(
        args["ln1_w"], args["ln1_b"], args["qkv_w"], args["qkv_b"],
        args["proj_w"], args["proj_b"], args["ln2_w"], args["ln2_b"],
        args["fc1_w"], args["fc1_b"], args["fc2_w"], args["fc2_b"])

    nc = _get_nc(nt, no, f_qk, f_f2)

    in_maps = []
    for core in range(N_CORES):
        b, g = divmod(core, 2)
        xr = np.roll(x[b], -g * no, axis=0)
        m = dict(shared)
        m["xT"] = np.ascontiguousarray(
            xr.T.reshape(DC, P, nt).transpose(1, 0, 2)).astype(NPBF16)
        rs = x[b, g * no:(g + 1) * no].T + pb[:, None]
        m["resid"] = np.ascontiguousarray(
            rs.reshape(DC, P, no).transpose(1, 0, 2)).astype(NPBF16)
        in_maps.append(m)

    res = run_bass_kernel_spmd(nc, in_maps, core_ids=list(range(N_CORES)),
                               trace=trace)
    out = np.empty((nb, nt, d), dtype=np.float32)
    for core in range(N_CORES):
        b, g = divmod(core, 2)
        o = res.results[core]["outT"]          # [P, DC, no]
        out[b, g * no:(g + 1) * no, :] = o.transpose(1, 0, 2).reshape(d, no).T
    return out, res


def kernel(**inputs) -> np.ndarray:
    out, _ = run_on_device(inputs, trace=False)
    return out


# revision 18
# speedup vs baseline: 1.0363x; 1.0161x over previous
"""Trainium2 Bass kernel for a dense transformer block.

Math (per batch element b of x[4, 2048, 768]):
    x = x + Attn(LN1(x));  x = x + MLP(LN2(x))   (12 heads, hidden 3072, exact gelu)

Sharding: 8 cores = (batch b in 0..3) x (sequence half g in 0..1). Each core
computes the full block for its own 1024 query tokens; k/v projections are
recomputed over the full 2048-token sequence of its batch element (no
collectives). Per-core token order is rotated so own tokens are always
columns 0..1023 -> one SPMD program for all cores.

Precision plan (validated off-line vs the fp32 reference, rel err ~9e-3
against a 2e-2 budget):
  - z1 (LN1 output), qkv/v weights: fp8 e4m3 -> DoubleRow matmuls (2x K rate).
    Weights are pre-scaled x16; the x16 on q and k folds into the softmax
    exp scale, the x16 on v cancels against a 16.0 "ones" column that also
    produces the softmax denominator.
  - softmax probabilities: fp8 e4m3, exp(s - 4.25) so the max stays ~110
    (fp8e4 max 240); the bias cancels in the normalization.
  - q/k activations, attn output, proj, LN2, fc1/fc2, h: bf16.
  - residual stream: bf16 in SBUF, f32 accumulation in PSUM, f32 output.

Attention structure per head pair (2hp, 2hp+1): score matmuls have K=64 so
the two heads run concurrently in the two row-halves of the PE array
(tile_position derives from base_partition). attn@v is a DoubleRow fp8
matmul with M=65 (64 v dims + denominator column). The denominator
reciprocal uses reciprocal_approx_fast and is broadcast across partitions
on the idle GPSIMD engine (no DRAM round-trip).

LayerNorm rstd is exp(-0.5*ln(var+eps)) so ACT only ever needs the
{exp,ln} table set plus gelu -- minimizes ACT_TABLE_LOAD switches.
"""

import sys

import numpy as np

sys.path.insert(0, "/opt/trn_rl_repo")

import ml_dtypes  # noqa: E402

import concourse.bacc as bacc  # noqa: E402
import concourse.mybir as mybir  # noqa: E402
import concourse.tile as tile  # noqa: E402
from concourse.bass_utils import run_bass_kernel_spmd  # noqa: E402

F32 = mybir.dt.float32
BF16 = mybir.dt.bfloat16
FP8 = mybir.dt.float8e4
AF = mybir.ActivationFunctionType
OP = mybir.AluOpType
DR = mybir.MatmulPerfMode.DoubleRow

P = 128
D = 768
DC = D // P            # 6 chunks of the model dim
H = 12
HD = 64
HID = 3072
HC = HID // P          # 24 chunks of the mlp hidden dim
EPS = 1e-5

W8 = 16.0              # fp8 weight pre-scale for qkv/v
EXP_BIAS = -4.25       # exp(s + EXP_BIAS); cancels in softmax
EXP_SCALE = (HD ** -0.5) / (W8 * W8)   # descales the x16 on q and x16 on k

NB = 4                 # batch
NT = 2048              # tokens per batch element (keys)
NO = NT // 2           # own tokens per core (queries)
N_CORES = 8

NPBF16 = ml_dtypes.bfloat16
NPFP8 = ml_dtypes.float8_e4m3


def _build_nc(nt, no, with_qk_bias, with_fc2_bias, reps=1, upto=99):
    """Build + schedule the SPMD Bass program (one core's view)."""
    nc = bacc.Bacc("TRN2", target_bir_lowering=False, debug=False,
                   num_devices=N_CORES)

    xT = nc.dram_tensor("xT", [P, DC, nt], BF16, kind="ExternalInput")
    resid = nc.dram_tensor("resid", [P, DC, no], BF16, kind="ExternalInput")
    wqkT = nc.dram_tensor("wqkT", [P, DC, 2 * D], FP8, kind="ExternalInput")
    wvT = nc.dram_tensor("wvT", [P, DC, D], FP8, kind="ExternalInput")
    pwT = nc.dram_tensor("pwT", [P, DC, D], BF16, kind="ExternalInput")
    w1T = nc.dram_tensor("w1T", [6, P, DC, 512], BF16, kind="ExternalInput")
    w2T = nc.dram_tensor("w2T", [6, P, HC, P], BF16, kind="ExternalInput")
    qk_bias = nc.dram_tensor("qk_bias", [P, 2 * DC], F32, kind="ExternalInput")
    b1p = nc.dram_tensor("b1p", [P, HC], F32, kind="ExternalInput")
    fc2_b = nc.dram_tensor("fc2_b", [P, DC], F32, kind="ExternalInput")
    outT = nc.dram_tensor("outT", [P, DC, no], F32, kind="ExternalOutput")
    dbg = {}
    if upto < 99:
        dbg["z1"] = nc.dram_tensor("dbg_z1", [P, DC, nt], FP8,
                                   kind="ExternalOutput")
        dbg["k"] = nc.dram_tensor("dbg_k", [P, DC, nt], BF16,
                                  kind="ExternalOutput")
        dbg["q"] = nc.dram_tensor("dbg_q", [P, DC, no], BF16,
                                  kind="ExternalOutput")
        dbg["v"] = nc.dram_tensor("dbg_v", [P, nt // 256, 2 * H, 80], FP8,
                                  kind="ExternalOutput")
        dbg["attn"] = nc.dram_tensor("dbg_attn", [P, DC, 512], BF16,
                                     kind="ExternalOutput")
        dbg["pT"] = nc.dram_tensor("dbg_pT", [P, nt // P, 512], FP8,
                                   kind="ExternalOutput")
        dbg["x1"] = nc.dram_tensor("dbg_x1", [P, DC, 512], BF16,
                                   kind="ExternalOutput")
        dbg["po"] = nc.dram_tensor("dbg_po", [P, 512], F32,
                                   kind="ExternalOutput")
        dbg["rec"] = nc.dram_tensor("dbg_rec", [1, 512], F32,
                                    kind="ExternalOutput")
        dbg["rbc"] = nc.dram_tensor("dbg_rbc", [HD, 512], F32,
                                    kind="ExternalOutput")

    with tile.TileContext(nc) as tc:
        for _ in range(reps):
            _emit(tc, nc, dict(
                xT=xT, resid=resid, wqkT=wqkT, wvT=wvT, pwT=pwT, w1T=w1T,
                w2T=w2T, qk_bias=qk_bias, b1p=b1p, fc2_b=fc2_b, outT=outT,
            ), nt, no, with_qk_bias, with_fc2_bias, upto, dbg)

    nc.compile()
    return nc


def _ln_stats(nc, ps_pool, tmps, ones_bf, x_src, sl_src, nmb, var_all,
              sl_dst):
    """Per-token -mean (bf16) and variance (f32) over the model dim for the
    512-token slice `sl_src` of channel-major bf16 x_src [P, DC, n], written
    to slice `sl_dst` of nmb/var_all. rstd finalization is separate so the
    ACT sqrt can be batched."""
    dc = DC
    # sum(x) over the model dim via ones-matmul (partition reduction)
    ps_s = ps_pool.tile([P, 512], F32, tag="mm")
    for c in range(dc):
        nc.tensor.matmul(ps_s, ones_bf, x_src[:, c, sl_src],
                         start=(c == 0), stop=(c == dc - 1))
    nc.vector.tensor_scalar_mul(nmb[:, sl_dst], ps_s, -1.0 / D)
    # sum(x^2): square on DVE into recycled bf16 tiles, then ones-matmul
    ps_q = ps_pool.tile([P, 512], F32, tag="mm")
    for c in range(dc):
        xsq = tmps.tile([P, 512], BF16, tag="xsq", bufs=3)
        nc.vector.tensor_mul(xsq, x_src[:, c, sl_src], x_src[:, c, sl_src])
        nc.tensor.matmul(ps_q, ones_bf, xsq,
                         start=(c == 0), stop=(c == dc - 1))
    var = var_all[:, sl_dst]
    nc.vector.tensor_scalar_mul(var, ps_q, 1.0 / D)
    msq = tmps.tile([P, 512], F32, tag="msq", bufs=1)
    nc.vector.tensor_mul(msq, nmb[:, sl_dst], nmb[:, sl_dst])
    nc.vector.tensor_tensor(var, var, msq, OP.subtract)


def _ln_rstd(nc, pool, eps_sb, var_all, rstdb, n, nm=""):
    """rstdb = bf16(1/sqrt(var+eps)) over the full [P, n] stats tile: one
    ACT Sqrt + one DVE approximate reciprocal + one cast."""
    sd = pool.tile([P, n], F32, tag=f"sd{n}", bufs=1, name=f"sd_{nm}{n}")
    nc.scalar.activation(sd, var_all[:, 0:n], AF.Sqrt, bias=eps_sb[:, 0:1])
    rf = pool.tile([P, n], F32, tag=f"rf{n}", bufs=1, name=f"rf_{nm}{n}")
    nc.vector.reciprocal_approx_fast(out=rf, in_=sd)
    nc.vector.tensor_copy(rstdb[:, 0:n], rf)


def _emit(tc, nc, io, nt, no, with_qk_bias, with_fc2_bias, upto=99, dbg=None):
    dc, hc = DC, HC
    ngk = nt // 512        # 512-wide groups over all tokens
    ngq = no // 512        # 512-wide groups over own tokens (pipeline stages)
    mt_n = nt // P         # 128-wide key tiles
    mtp_n = mt_n // 2      # pairs of key tiles (DoubleRow K chunks)
    _stack = []

    def _pool(*a, **k):
        p = tc.alloc_tile_pool(*a, **k)
        _stack.append(p)
        return p

    def _rel(p):
        assert _stack[-1] is p
        _stack.pop()
        p.release()

    def _cut():
        for p in reversed(_stack):
            p.release()
        _stack.clear()

    # ---- long-lived pools (created first, released last) -------------------
    consts = _pool(name="consts", bufs=1)
    tmps = _pool(name="tmps", bufs=2)
    ps_mm = _pool(name="ps_mm", bufs=2, space="PSUM")
    ps_sc = _pool(name="ps_sc", bufs=2, space="PSUM")
    ps_po = _pool(name="ps_po", bufs=2, space="PSUM")
    p_kT = _pool(name="p_kT", bufs=1)
    p_qT = _pool(name="p_qT", bufs=1)
    p_v = _pool(name="p_v", bufs=1)
    p_rec = _pool(name="p_rec", bufs=2)

    ones_bf = consts.tile([P, P], BF16)
    nc.vector.memset(ones_bf, 1.0)
    eps_sb = consts.tile([P, 1], F32)
    nc.vector.memset(eps_sb, EPS)
    zero_sb = consts.tile([P, 1], F32)
    nc.vector.memset(zero_sb, 0.0)
    expb_sb = consts.tile([P, 1], F32)
    nc.vector.memset(expb_sb, EXP_BIAS)
    one_sb = consts.tile([P, 1], F32)
    nc.vector.memset(one_sb, 1.0)
    qkb_sb = consts.tile([P, 2 * dc], F32)
    nc.sync.dma_start(qkb_sb, io["qk_bias"][:, :])
    b1p_sb = consts.tile([P, hc], F32)
    nc.sync.dma_start(b1p_sb, io["b1p"][:, :])
    fc2b_sb = consts.tile([P, dc], F32)
    nc.sync.dma_start(fc2b_sb, io["fc2_b"][:, :])

    kT = p_kT.tile([P, dc, nt], BF16)
    qT = p_qT.tile([P, dc, no], BF16)
    # v: [P, key-tile-pair, 2*head + tile-in-pair, 80]; col 64 = 16.0 for the
    # softmax denominator, cols 65..79 pad the fp8 DoubleRow stride to 80.
    v_pair = p_v.tile([P, mtp_n, 2 * H, 80], FP8)

    # ---- phase-1/2 pools (released before attention pools are created) -----
    p_wqk = _pool(name="p_wqk", bufs=1)
    p_z1 = _pool(name="p_z1", bufs=1)
    p_st1 = _pool(name="p_st1", bufs=1)
    p_xT = _pool(name="p_xT", bufs=1)

    wqk_sb = p_wqk.tile([P, dc, 2 * D], FP8, tag="wqk")
    nc.sync.dma_start(wqk_sb, io["wqkT"][:, :, :])
    wv_sb = p_wqk.tile([P, dc, D], FP8, tag="wv")
    nc.sync.dma_start(wv_sb, io["wvT"][:, :, :])

    xT_sb = p_xT.tile([P, dc, nt], BF16)
    for c in range(dc):
        for hh in range(nt // 1024):
            hsl = slice(hh * 1024, (hh + 1) * 1024)
            nc.sync.dma_start(xT_sb[:, c, hsl], io["xT"][:, c, hsl])

    nmb = p_st1.tile([P, nt], BF16, tag="nmb")
    rstdb = p_st1.tile([P, nt], BF16, tag="rstdb")
    var1 = p_st1.tile([P, nt], F32, tag="var1")
    z1T = p_z1.tile([P, dc, nt], FP8)

    # ---------------- Phase 1: LN1 stats + z1 (fp8) -------------------------
    for ng in range(ngk):
        sl = slice(ng * 512, (ng + 1) * 512)
        _ln_stats(nc, ps_mm, tmps, ones_bf, xT_sb, sl, nmb, var1, sl)
    _ln_rstd(nc, p_st1, eps_sb, var1, rstdb, nt)
    for ng in range(ngk):
        sl = slice(ng * 512, (ng + 1) * 512)
        for c in range(dc):
            t = tmps.tile([P, 512], BF16, tag="lnt", bufs=3)
            nc.vector.tensor_add(t, xT_sb[:, c, sl], nmb[:, sl])
            nc.vector.tensor_mul(z1T[:, c, sl], t, rstdb[:, sl])
    _rel(p_xT)
    _rel(p_st1)
    if upto <= 1:
        nc.sync.dma_start(dbg["z1"][:, :, :], z1T)
        _cut()
        return

    # ---------------- Phase 2: qkv projections (fp8 DoubleRow) --------------
    # k and q chunk-by-chunk so attention on head pair hp can start as soon
    # as chunk hp is done.
    for cc in range(dc):
        for ng in range(ngk):
            sl = slice(ng * 512, (ng + 1) * 512)
            ps = ps_mm.tile([P, 512], F32, tag="mm")
            for c in range(3):
                nc.tensor.matmul(
                    ps, wqk_sb[:, 2 * c:2 * c + 2,
                               D + cc * P:D + (cc + 1) * P],
                    z1T[:, 2 * c:2 * c + 2, sl],
                    start=(c == 0), stop=(c == 2), perf_mode=DR)
            if with_qk_bias:
                nc.vector.tensor_scalar(kT[:, cc, sl], ps,
                                        qkb_sb[:, dc + cc:dc + cc + 1], None,
                                        OP.add)
            else:
                nc.vector.tensor_copy(kT[:, cc, sl], ps)
        for g in range(ngq):
            sl = slice(g * 512, (g + 1) * 512)
            ps = ps_mm.tile([P, 512], F32, tag="mm")
            for c in range(3):
                nc.tensor.matmul(
                    ps, wqk_sb[:, 2 * c:2 * c + 2, cc * P:(cc + 1) * P],
                    z1T[:, 2 * c:2 * c + 2, sl],
                    start=(c == 0), stop=(c == 2), perf_mode=DR)
            if with_qk_bias:
                nc.vector.tensor_scalar(qT[:, cc, sl], ps,
                                        qkb_sb[:, cc:cc + 1], None, OP.add)
            else:
                nc.vector.tensor_copy(qT[:, cc, sl], ps)

    # v (token-major): half 0 first so heads 0..5 complete early.
    nc.vector.memset(v_pair[:, :, :, 64:65], W8)
    for half in range(2):
        rhs_sl = slice(half * 384, (half + 1) * 384)
        for mt in range(mt_n):
            ps = ps_mm.tile([P, 384], F32, tag="mm")
            for c in range(3):
                nc.tensor.matmul(
                    ps, z1T[:, 2 * c:2 * c + 2, mt * P:(mt + 1) * P],
                    wv_sb[:, 2 * c:2 * c + 2, rhs_sl],
                    start=(c == 0), stop=(c == 2), perf_mode=DR)
            dst = v_pair[:, mt // 2,
                         slice(12 * half + (mt % 2), 12 * half + 12, 2), 0:64]
            nc.vector.tensor_copy(
                dst, ps.rearrange("p (h d) -> p h d", d=64))

    if upto <= 2:
        nc.sync.dma_start(dbg["z1"][:, :, :], z1T)
        nc.sync.dma_start(dbg["k"][:, :, :], kT)
        nc.sync.dma_start(dbg["q"][:, :, :], qT)
        nc.sync.dma_start(dbg["v"][:, :, :, :], v_pair)
        _rel(p_z1)
        _rel(p_wqk)
        _cut()
        return
    _rel(p_z1)
    _rel(p_wqk)

    # ---- attention/MLP pools (reuse the phase-1/2 SBUF) --------------------
    p_pT = _pool(name="p_pT", bufs=3)
    p_attn = _pool(name="p_attn", bufs=2)
    p_x1 = _pool(name="p_x1", bufs=1)
    p_z2 = _pool(name="p_z2", bufs=1)
    p_h = _pool(name="p_h", bufs=1)
    p_w12 = _pool(name="p_w12", bufs=2)
    p_pw = _pool(name="p_pw", bufs=1)
    p_res = _pool(name="p_res", bufs=1)
    p_st2 = _pool(name="p_st2", bufs=2)
    p_x2 = _pool(name="p_x2", bufs=2)

    pw_sb = p_pw.tile([P, dc, D], BF16)
    nc.sync.dma_start(pw_sb, io["pwT"][:, :, :])
    x1T = p_x1.tile([P, dc, no], BF16)

    def attention(g, attnT, pairs):
        sl = slice(g * 512, (g + 1) * 512)
        for hp in pairs:
            pTs = [p_pT.tile([P, mt_n, 512], FP8, tag="pT",
                             name=f"pT_{g}_{2 * hp + s}") for s in range(2)]
            for mtp in range(mtp_n):
                scs = [ps_sc.tile([P, 2, 512], F32, tag="sc",
                                  name=f"sc_{g}_{hp}_{mtp}_{s}")
                       for s in range(2)]
                for s2 in range(2):
                    mt = 2 * mtp + s2
                    msl = slice(mt * P, (mt + 1) * P)
                    # two heads run concurrently in the two row-halves
                    nc.tensor.matmul(scs[0][:, s2], kT[0:HD, hp, msl],
                                     qT[0:HD, hp, sl])
                    nc.tensor.matmul(scs[1][:, s2], kT[HD:P, hp, msl],
                                     qT[HD:P, hp, sl])
                for s in range(2):
                    nc.scalar.activation(pTs[s][:, 2 * mtp:2 * mtp + 2],
                                         scs[s][:, :, :], AF.Exp,
                                         bias=expb_sb[:, 0:1],
                                         scale=EXP_SCALE)
            if dbg and upto <= 3 and g == 0 and hp == 0:
                nc.sync.dma_start(dbg["pT"][:, :, :], pTs[0])
            for s in range(2):
                h = 2 * hp + s
                po = ps_po.tile([P, 512], F32, tag="po")
                for mtp in range(mtp_n):
                    nc.tensor.matmul(po[0:65],
                                     v_pair[:, mtp, 2 * h:2 * h + 2, 0:65],
                                     pTs[s][:, 2 * mtp:2 * mtp + 2, :],
                                     start=(mtp == 0), stop=(mtp == mtp_n - 1),
                                     perf_mode=DR)
                den = p_rec.tile([1, 512], F32, tag="den", bufs=1)
                nc.vector.tensor_copy(den, po[64:65, :])
                rec = p_rec.tile([1, 512], F32, tag="rec", bufs=1)
                nc.vector.reciprocal_approx_fast(out=rec, in_=den)
                rbc = p_rec.tile([HD, 512], F32, tag="rbc", bufs=2)
                nc.gpsimd.partition_broadcast(rbc, rec, channels=HD)
                if dbg and upto <= 3 and g == 0 and h == 0:
                    po_d = tmps.tile([P, 512], F32, tag="po_d", bufs=1)
                    nc.vector.tensor_copy(po_d, po)
                    nc.sync.dma_start(dbg["po"][:, :], po_d)
                    nc.sync.dma_start(dbg["rec"][:, :], rec)
                    nc.sync.dma_start(dbg["rbc"][:, :], rbc)
                nc.vector.tensor_tensor(attnT[s * HD:(s + 1) * HD, hp, :],
                                        po[0:HD, :], rbc, OP.mult)

    def proj_ln2(g, attnT):
        sl = slice(g * 512, (g + 1) * 512)
        res_sb = p_res.tile([P, dc, 512], BF16, tag="res", bufs=1,
                    name=f"res_{g}")
        for c in range(dc):
            nc.sync.dma_start(res_sb[:, c], io["resid"][:, c, sl])
        for ec in range(dc):
            ps = ps_mm.tile([P, 512], F32, tag="mm")
            for c in range(dc):
                nc.tensor.matmul(ps, pw_sb[:, c, ec * P:(ec + 1) * P],
                                 attnT[:, c, :], start=(c == 0),
                                 stop=(c == dc - 1))
            nc.vector.tensor_add(x1T[:, ec, sl], ps, res_sb[:, ec])
        # LN2 on this 512-token group
        nm2 = p_st2.tile([P, 512], BF16, tag="nm2", bufs=1,
                 name=f"nm2_{g}")
        rstd2 = p_st2.tile([P, 512], BF16, tag="rstd2", bufs=1,
                   name=f"rstd2_{g}")
        var2 = p_st2.tile([P, 512], F32, tag="var2", bufs=1,
                  name=f"var2_{g}")
        _ln_stats(nc, ps_mm, tmps, ones_bf, x1T, sl, nm2, var2, slice(0, 512))
        _ln_rstd(nc, p_st2, eps_sb, var2, rstd2, 512, nm=f"{g}_")
        z2T = p_z2.tile([P, dc, 512], BF16, tag="z2", name=f"z2_{g}")
        for c in range(dc):
            t = tmps.tile([P, 512], BF16, tag="lnt", bufs=3)
            nc.vector.tensor_add(t, x1T[:, c, sl], nm2)
            nc.vector.tensor_mul(z2T[:, c], t, rstd2)
        return z2T

    def fc1_slices(g, z2T, hT, slices, exp_gelu=False):
        for i in slices:
            w1s = p_w12.tile([P, dc, 512], BF16, tag="w1", name=f"w1_{g}_{i}")
            nc.sync.dma_start(w1s, io["w1T"][i])
            for ci in range(4):
                cc = i * 4 + ci
                ps = ps_mm.tile([P, 512], F32, tag="mm")
                for c in range(dc):
                    nc.tensor.matmul(ps, w1s[:, c, ci * P:(ci + 1) * P],
                                     z2T[:, c], start=(c == 0),
                                     stop=(c == dc - 1))
                if exp_gelu:
                    # gelu(x) ~= x*sigmoid(1.702x) using the exp table set
                    # (no ACT table switch inside the attention exp window;
                    # requires zero fc1 bias).
                    e = tmps.tile([P, 512], F32, tag="ge", bufs=2,
                                  name=f"ge_{g}_{cc}")
                    nc.scalar.activation(e, ps, AF.Exp,
                                         bias=zero_sb[:, 0:1], scale=-1.702)
                    nc.vector.tensor_scalar(e, e, one_sb[:, 0:1],
                                            None, OP.add)
                    r = tmps.tile([P, 512], F32, tag="gr", bufs=2,
                                  name=f"gr_{g}_{cc}")
                    nc.vector.reciprocal_approx_fast(out=r, in_=e)
                    nc.vector.tensor_tensor(hT[:, cc], ps, r, OP.mult)
                else:
                    nc.scalar.activation(hT[:, cc], ps, AF.Gelu,
                                         bias=b1p_sb[:, cc:cc + 1])

    def fc2(g, hT):
        sl = slice(g * 512, (g + 1) * 512)
        for ec in range(dc):
            w2s = p_w12.tile([P, hc, P], BF16, tag="w2", name=f"w2_{g}_{ec}")
            nc.sync.dma_start(w2s, io["w2T"][ec])
            ps = ps_mm.tile([P, 512], F32, tag="mm")
            for c in range(hc):
                nc.tensor.matmul(ps, w2s[:, c], hT[:, c],
                                 start=(c == 0), stop=(c == hc - 1))
            x2 = p_x2.tile([P, 512], F32, tag="x2")
            if with_fc2_bias:
                t = tmps.tile([P, 512], F32, tag="f2t")
                nc.vector.tensor_scalar(t, ps, fc2b_sb[:, ec:ec + 1], None,
                                        OP.add)
                nc.vector.tensor_add(x2, t, x1T[:, ec, sl])
            else:
                nc.vector.tensor_add(x2, ps, x1T[:, ec, sl])
            nc.sync.dma_start(io["outT"][:, ec, sl], x2)

    # pipeline: attention(g1) emission is interleaved with fc1(g0) in
    # chunks so the exp-bound attention window keeps the tensor engine fed
    # (and the HAM clock-gate warm) with MLP matmuls; fc2(g0) follows to
    # fill the tail of the exp window.
    attnT0 = p_attn.tile([P, dc, 512], BF16, tag="attnT", name="attnT_0")
    attention(0, attnT0, range(dc))
    if upto <= 3:
        nc.sync.dma_start(dbg["attn"][:, :, :], attnT0)
        _cut()
        return
    z2_0 = proj_ln2(0, attnT0)
    if upto <= 4:
        nc.sync.dma_start(dbg["x1"][:, :, :], x1T[:, :, 0:512])
        nc.sync.dma_start(dbg["attn"][:, :, :], attnT0)
        _cut()
        return
    attnT1 = p_attn.tile([P, dc, 512], BF16, tag="attnT", name="attnT_1")
    hT0 = p_h.tile([P, hc, 512], BF16, tag="hT", name="hT_0")
    for chunk in range(3):
        attention(1, attnT1, [2 * chunk, 2 * chunk + 1])
        fc1_slices(0, z2_0, hT0, [2 * chunk, 2 * chunk + 1],
                   exp_gelu=not with_qk_bias)
    fc2(0, hT0)
    z2_1 = proj_ln2(1, attnT1)
    hT1 = p_h.tile([P, hc, 512], BF16, tag="hT", name="hT_1")
    fc1_slices(1, z2_1, hT1, range(6))
    fc2(1, hT1)
    _cut()


# --------------------------------------------------------------------------
# Host side
# --------------------------------------------------------------------------

_NC_CACHE = {}


def _get_nc(nt, no, with_qk_bias, with_fc2_bias, reps=1, upto=99):
    key = (nt, no, with_qk_bias, with_fc2_bias, reps, upto)
    if key not in _NC_CACHE:
        _NC_CACHE[key] = _build_nc(nt, no, with_qk_bias, with_fc2_bias, reps,
                                   upto)
    return _NC_CACHE[key]


def _prep_weights(ln1_w, ln1_b, qkv_w, qkv_b, proj_w, proj_b,
                  ln2_w, ln2_b, fc1_w, fc1_b, fc2_w, fc2_b):
    w_qkv = qkv_w * ln1_w[None, :]
    b_qkv = qkv_w @ ln1_b + qkv_b
    pb = proj_b + proj_w @ b_qkv[2 * D:]
    w1 = fc1_w * ln2_w[None, :]
    b1p = fc1_b + fc1_w @ ln2_b

    def col(v, chunks):
        return np.ascontiguousarray(v.reshape(chunks, P).T.astype(np.float32))

    def sb(wT, chunks, npdt):
        # [K, M] -> [P, chunks, M] with K = chunks*P (SBUF layout)
        k, m = wT.shape
        return np.ascontiguousarray(
            wT.reshape(chunks, P, m).transpose(1, 0, 2).astype(npdt))

    w1_s = sb(w1.T, DC, NPBF16)                   # [P, DC, 3072]
    w2_s = sb(fc2_w.T, HC, NPBF16)                # [P, HC, 768]
    shared = {
        "wqkT": sb(w_qkv[:2 * D].T * W8, DC, NPFP8),
        "wvT": sb(w_qkv[2 * D:].T * W8, DC, NPFP8),
        "pwT": sb(proj_w.T, DC, NPBF16),
        "w1T": np.ascontiguousarray(
            w1_s.reshape(P, DC, 6, 512).transpose(2, 0, 1, 3)),
        "w2T": np.ascontiguousarray(
            w2_s.reshape(P, HC, 6, P).transpose(2, 0, 1, 3)),
        "qk_bias": col(b_qkv[:2 * D] * W8, 2 * DC),
        "b1p": col(b1p, HC),
        "fc2_b": col(fc2_b, DC),
    }
    flags = (bool(np.any(b_qkv[:2 * D])) or bool(np.any(b1p)),
             bool(np.any(fc2_b)))
    return shared, pb, flags


def run_on_device(inputs, trace=False):
    x = np.asarray(inputs["x"], dtype=np.float32)
    nb, nt, d = x.shape
    no = nt // 2
    args = {k: np.asarray(v, dtype=np.float32) for k, v in inputs.items()
            if k != "x"}
    shared, pb, (f_qk, f_f2) = _prep_weights(
        args["ln1_w"], args["ln1_b"], args["qkv_w"], args["qkv_b"],
        args["proj_w"], args["proj_b"], args["ln2_w"], args["ln2_b"],
        args["fc1_w"], args["fc1_b"], args["fc2_w"], args["fc2_b"])

    nc = _get_nc(nt, no, f_qk, f_f2)

    in_maps = []
    for core in range(N_CORES):
        b, g = divmod(core, 2)
        xr = np.roll(x[b], -g * no, axis=0)
        m = dict(shared)
        m["xT"] = np.ascontiguousarray(
            xr.T.reshape(DC, P, nt).transpose(1, 0, 2)).astype(NPBF16)
        rs = x[b, g * no:(g + 1) * no].T + pb[:, None]
        m["resid"] = np.ascontiguousarray(
            rs.reshape(DC, P, no).transpose(1, 0, 2)).astype(NPBF16)
        in_maps.append(m)

    res = run_bass_kernel_spmd(nc, in_maps, core_ids=list(range(N_CORES)),
                               trace=trace)
    out = np.empty((nb, nt, d), dtype=np.float32)
    for core in range(N_CORES):
        b, g = divmod(core, 2)
        o = res.results[core]["outT"]          # [P, DC, no]
        out[b, g * no:(g + 1) * no, :] = o.transpose(1, 0, 2).reshape(d, no).T
    return out, res


def kernel(**inputs) -> np.ndarray:
    out, _ = run_on_device(inputs, trace=False)
    return out


# revision 19
# speedup vs baseline: 1.0735x; 1.0358x over previous
"""Trainium2 Bass kernel for a dense transformer block.

Math (per batch element b of x[4, 2048, 768]):
    x = x + Attn(LN1(x));  x = x + MLP(LN2(x))   (12 heads, hidden 3072, exact gelu)

Sharding: 8 cores = (batch b in 0..3) x (sequence half g in 0..1). Each core
computes the full block for its own 1024 query tokens; k/v projections are
recomputed over the full 2048-token sequence of its batch element (no
collectives). Per-core token order is rotated so own tokens are always
columns 0..1023 -> one SPMD program for all cores.

Precision plan (validated off-line vs the fp32 reference, rel err ~9e-3
against a 2e-2 budget):
  - z1 (LN1 output), qkv/v weights: fp8 e4m3 -> DoubleRow matmuls (2x K rate).
    Weights are pre-scaled x16; the x16 on q and k folds into the softmax
    exp scale, the x16 on v cancels against a 16.0 "ones" column that also
    produces the softmax denominator.
  - softmax probabilities: fp8 e4m3, exp(s - 4.25) so the max stays ~110
    (fp8e4 max 240); the bias cancels in the normalization.
  - q/k activations, attn output, proj, LN2, fc1/fc2, h: bf16.
  - residual stream: bf16 in SBUF, f32 accumulation in PSUM, f32 output.

Attention structure per head pair (2hp, 2hp+1): score matmuls have K=64 so
the two heads run concurrently in the two row-halves of the PE array
(tile_position derives from base_partition). attn@v is a DoubleRow fp8
matmul with M=65 (64 v dims + denominator column). The denominator
reciprocal uses reciprocal_approx_fast and is broadcast across partitions
on the idle GPSIMD engine (no DRAM round-trip).

LayerNorm rstd is exp(-0.5*ln(var+eps)) so ACT only ever needs the
{exp,ln} table set plus gelu -- minimizes ACT_TABLE_LOAD switches.
"""

import sys

import numpy as np

sys.path.insert(0, "/opt/trn_rl_repo")

import ml_dtypes  # noqa: E402

import concourse.bacc as bacc  # noqa: E402
import concourse.mybir as mybir  # noqa: E402
import concourse.tile as tile  # noqa: E402
from concourse.bass_utils import run_bass_kernel_spmd  # noqa: E402

F32 = mybir.dt.float32
BF16 = mybir.dt.bfloat16
FP8 = mybir.dt.float8e4
AF = mybir.ActivationFunctionType
OP = mybir.AluOpType
DR = mybir.MatmulPerfMode.DoubleRow

P = 128
D = 768
DC = D // P            # 6 chunks of the model dim
H = 12
HD = 64
HID = 3072
HC = HID // P          # 24 chunks of the mlp hidden dim
EPS = 1e-5

W8 = 16.0              # fp8 weight pre-scale for qkv/v
EXP_BIAS = -4.25       # exp(s + EXP_BIAS); cancels in softmax
EXP_SCALE = (HD ** -0.5) / (W8 * W8)   # descales the x16 on q and x16 on k

NB = 4                 # batch
NT = 2048              # tokens per batch element (keys)
NO = NT // 2           # own tokens per core (queries)
N_CORES = 8

NPBF16 = ml_dtypes.bfloat16
NPFP8 = ml_dtypes.float8_e4m3


def _build_nc(nt, no, with_qk_bias, with_fc2_bias, reps=1, upto=99):
    """Build + schedule the SPMD Bass program (one core's view)."""
    nc = bacc.Bacc("TRN2", target_bir_lowering=False, debug=False,
                   num_devices=N_CORES)

    xT = nc.dram_tensor("xT", [P, DC, nt], BF16, kind="ExternalInput")
    resid = nc.dram_tensor("resid", [P, DC, no], BF16, kind="ExternalInput")
    wqkT = nc.dram_tensor("wqkT", [P, DC, 2 * D], FP8, kind="ExternalInput")
    wvT = nc.dram_tensor("wvT", [P, DC, D], FP8, kind="ExternalInput")
    pwT = nc.dram_tensor("pwT", [P, DC, D], BF16, kind="ExternalInput")
    w1T = nc.dram_tensor("w1T", [6, P, DC, 512], BF16, kind="ExternalInput")
    w2T = nc.dram_tensor("w2T", [6, P, HC, P], BF16, kind="ExternalInput")
    qk_bias = nc.dram_tensor("qk_bias", [P, 2 * DC], F32, kind="ExternalInput")
    b1p = nc.dram_tensor("b1p", [P, HC], F32, kind="ExternalInput")
    fc2_b = nc.dram_tensor("fc2_b", [P, DC], F32, kind="ExternalInput")
    outT = nc.dram_tensor("outT", [P, DC, no], F32, kind="ExternalOutput")
    dbg = {}
    if upto < 99:
        dbg["z1"] = nc.dram_tensor("dbg_z1", [P, DC, nt], FP8,
                                   kind="ExternalOutput")
        dbg["k"] = nc.dram_tensor("dbg_k", [P, DC, nt], BF16,
                                  kind="ExternalOutput")
        dbg["q"] = nc.dram_tensor("dbg_q", [P, DC, no], BF16,
                                  kind="ExternalOutput")
        dbg["v"] = nc.dram_tensor("dbg_v", [P, nt // 256, 2 * H, 80], FP8,
                                  kind="ExternalOutput")
        dbg["attn"] = nc.dram_tensor("dbg_attn", [P, DC, 512], BF16,
                                     kind="ExternalOutput")
        dbg["pT"] = nc.dram_tensor("dbg_pT", [P, nt // P, 512], FP8,
                                   kind="ExternalOutput")
        dbg["x1"] = nc.dram_tensor("dbg_x1", [P, DC, 512], BF16,
                                   kind="ExternalOutput")
        dbg["po"] = nc.dram_tensor("dbg_po", [P, 512], F32,
                                   kind="ExternalOutput")
        dbg["rec"] = nc.dram_tensor("dbg_rec", [1, 512], F32,
                                    kind="ExternalOutput")
        dbg["rbc"] = nc.dram_tensor("dbg_rbc", [HD, 512], F32,
                                    kind="ExternalOutput")

    with tile.TileContext(nc) as tc:
        for _ in range(reps):
            _emit(tc, nc, dict(
                xT=xT, resid=resid, wqkT=wqkT, wvT=wvT, pwT=pwT, w1T=w1T,
                w2T=w2T, qk_bias=qk_bias, b1p=b1p, fc2_b=fc2_b, outT=outT,
            ), nt, no, with_qk_bias, with_fc2_bias, upto, dbg)

    nc.compile()
    return nc


def _ln_stats(nc, ps_pool, tmps, ones_bf, x_src, sl_src, nmb, var_all,
              sl_dst):
    """Per-token -mean (bf16) and variance (f32) over the model dim for the
    512-token slice `sl_src` of channel-major bf16 x_src [P, DC, n], written
    to slice `sl_dst` of nmb/var_all. rstd finalization is separate so the
    ACT sqrt can be batched."""
    dc = DC
    # sum(x) over the model dim via ones-matmul (partition reduction)
    ps_s = ps_pool.tile([P, 512], F32, tag="mm")
    for c in range(dc):
        nc.tensor.matmul(ps_s, ones_bf, x_src[:, c, sl_src],
                         start=(c == 0), stop=(c == dc - 1))
    nc.vector.tensor_scalar_mul(nmb[:, sl_dst], ps_s, -1.0 / D)
    # sum(x^2): square on DVE into recycled bf16 tiles, then ones-matmul
    ps_q = ps_pool.tile([P, 512], F32, tag="mm")
    for c in range(dc):
        xsq = tmps.tile([P, 512], BF16, tag="xsq", bufs=3)
        nc.vector.tensor_mul(xsq, x_src[:, c, sl_src], x_src[:, c, sl_src])
        nc.tensor.matmul(ps_q, ones_bf, xsq,
                         start=(c == 0), stop=(c == dc - 1))
    var = var_all[:, sl_dst]
    nc.vector.tensor_scalar_mul(var, ps_q, 1.0 / D)
    msq = tmps.tile([P, 512], F32, tag="msq", bufs=1)
    nc.vector.tensor_mul(msq, nmb[:, sl_dst], nmb[:, sl_dst])
    nc.vector.tensor_tensor(var, var, msq, OP.subtract)


def _ln_rstd(nc, pool, eps_sb, var_all, rstdb, n, nm=""):
    """rstdb = bf16(1/sqrt(var+eps)) over the full [P, n] stats tile: one
    ACT Sqrt + one DVE approximate reciprocal + one cast."""
    sd = pool.tile([P, n], F32, tag=f"sd{n}", bufs=1, name=f"sd_{nm}{n}")
    nc.scalar.activation(sd, var_all[:, 0:n], AF.Sqrt, bias=eps_sb[:, 0:1])
    rf = pool.tile([P, n], F32, tag=f"rf{n}", bufs=1, name=f"rf_{nm}{n}")
    nc.vector.reciprocal_approx_fast(out=rf, in_=sd)
    nc.vector.tensor_copy(rstdb[:, 0:n], rf)


def _emit(tc, nc, io, nt, no, with_qk_bias, with_fc2_bias, upto=99, dbg=None):
    dc, hc = DC, HC
    ngk = nt // 512        # 512-wide groups over all tokens
    ngq = no // 512        # 512-wide groups over own tokens (pipeline stages)
    mt_n = nt // P         # 128-wide key tiles
    mtp_n = mt_n // 2      # pairs of key tiles (DoubleRow K chunks)
    _stack = []

    def _pool(*a, **k):
        p = tc.alloc_tile_pool(*a, **k)
        _stack.append(p)
        return p

    def _rel(p):
        assert _stack[-1] is p
        _stack.pop()
        p.release()

    def _cut():
        for p in reversed(_stack):
            p.release()
        _stack.clear()

    # ---- long-lived pools (created first, released last) -------------------
    consts = _pool(name="consts", bufs=1)
    tmps = _pool(name="tmps", bufs=2)
    ps_mm = _pool(name="ps_mm", bufs=2, space="PSUM")
    ps_sc = _pool(name="ps_sc", bufs=2, space="PSUM")
    ps_po = _pool(name="ps_po", bufs=2, space="PSUM")
    p_kT = _pool(name="p_kT", bufs=1)
    p_qT = _pool(name="p_qT", bufs=1)
    p_v = _pool(name="p_v", bufs=1)
    p_rec = _pool(name="p_rec", bufs=2)

    ones_bf = consts.tile([P, P], BF16)
    nc.vector.memset(ones_bf, 1.0)
    eps_sb = consts.tile([P, 1], F32)
    nc.vector.memset(eps_sb, EPS)
    zero_sb = consts.tile([P, 1], F32)
    nc.vector.memset(zero_sb, 0.0)
    expb_sb = consts.tile([P, 1], F32)
    nc.vector.memset(expb_sb, EXP_BIAS)
    one_sb = consts.tile([P, 1], F32)
    nc.vector.memset(one_sb, 1.0)
    qkb_sb = consts.tile([P, 2 * dc], F32)
    nc.sync.dma_start(qkb_sb, io["qk_bias"][:, :])
    b1p_sb = consts.tile([P, hc], F32)
    nc.sync.dma_start(b1p_sb, io["b1p"][:, :])
    fc2b_sb = consts.tile([P, dc], F32)
    nc.sync.dma_start(fc2b_sb, io["fc2_b"][:, :])

    kT = p_kT.tile([P, dc, nt], BF16)
    qT = p_qT.tile([P, dc, no], BF16)
    # v: [P, key-tile-pair, 2*head + tile-in-pair, 80]; col 64 = 16.0 for the
    # softmax denominator, cols 65..79 pad the fp8 DoubleRow stride to 80.
    v_pair = p_v.tile([P, mtp_n, 2 * H, 80], FP8)

    # ---- phase-1/2 pools (released before attention pools are created) -----
    p_wqk = _pool(name="p_wqk", bufs=1)
    p_z1 = _pool(name="p_z1", bufs=1)
    p_st1 = _pool(name="p_st1", bufs=1)
    p_xT = _pool(name="p_xT", bufs=1)

    wqk_sb = p_wqk.tile([P, dc, 2 * D], FP8, tag="wqk")
    nc.sync.dma_start(wqk_sb, io["wqkT"][:, :, :])
    wv_sb = p_wqk.tile([P, dc, D], FP8, tag="wv")
    nc.sync.dma_start(wv_sb, io["wvT"][:, :, :])

    xT_sb = p_xT.tile([P, dc, nt], BF16)
    for c in range(dc):
        for hh in range(nt // 1024):
            hsl = slice(hh * 1024, (hh + 1) * 1024)
            nc.sync.dma_start(xT_sb[:, c, hsl], io["xT"][:, c, hsl])

    nmb = p_st1.tile([P, nt], BF16, tag="nmb")
    rstdb = p_st1.tile([P, nt], BF16, tag="rstdb")
    var1 = p_st1.tile([P, nt], F32, tag="var1")
    z1T = p_z1.tile([P, dc, nt], FP8)

    # ---------------- Phase 1: LN1 stats + z1 (fp8) -------------------------
    for ng in range(ngk):
        sl = slice(ng * 512, (ng + 1) * 512)
        _ln_stats(nc, ps_mm, tmps, ones_bf, xT_sb, sl, nmb, var1, sl)
        sd = p_st1.tile([P, 512], F32, tag="sd1", bufs=2, name=f"sd1_{ng}")
        nc.scalar.activation(sd, var1[:, sl], AF.Sqrt, bias=eps_sb[:, 0:1])
        rf = p_st1.tile([P, 512], F32, tag="rf1", bufs=2, name=f"rf1_{ng}")
        nc.vector.reciprocal_approx_fast(out=rf, in_=sd)
        nc.vector.tensor_copy(rstdb[:, sl], rf)
        for c in range(dc):
            t = tmps.tile([P, 512], BF16, tag="lnt", bufs=3)
            nc.vector.tensor_add(t, xT_sb[:, c, sl], nmb[:, sl])
            nc.vector.tensor_mul(z1T[:, c, sl], t, rstdb[:, sl])
    _rel(p_xT)
    _rel(p_st1)
    if upto <= 1:
        nc.sync.dma_start(dbg["z1"][:, :, :], z1T)
        _cut()
        return

    # ---------------- Phase 2: qkv projections (fp8 DoubleRow) --------------
    # k and q chunk-by-chunk so attention on head pair hp can start as soon
    # as chunk hp is done.
    for cc in range(dc):
        for ng in range(ngk):
            sl = slice(ng * 512, (ng + 1) * 512)
            ps = ps_mm.tile([P, 512], F32, tag="mm")
            for c in range(3):
                nc.tensor.matmul(
                    ps, wqk_sb[:, 2 * c:2 * c + 2,
                               D + cc * P:D + (cc + 1) * P],
                    z1T[:, 2 * c:2 * c + 2, sl],
                    start=(c == 0), stop=(c == 2), perf_mode=DR)
            if with_qk_bias:
                nc.vector.tensor_scalar(kT[:, cc, sl], ps,
                                        qkb_sb[:, dc + cc:dc + cc + 1], None,
                                        OP.add)
            else:
                nc.vector.tensor_copy(kT[:, cc, sl], ps)
        for g in range(ngq):
            sl = slice(g * 512, (g + 1) * 512)
            ps = ps_mm.tile([P, 512], F32, tag="mm")
            for c in range(3):
                nc.tensor.matmul(
                    ps, wqk_sb[:, 2 * c:2 * c + 2, cc * P:(cc + 1) * P],
                    z1T[:, 2 * c:2 * c + 2, sl],
                    start=(c == 0), stop=(c == 2), perf_mode=DR)
            if with_qk_bias:
                nc.vector.tensor_scalar(qT[:, cc, sl], ps,
                                        qkb_sb[:, cc:cc + 1], None, OP.add)
            else:
                nc.vector.tensor_copy(qT[:, cc, sl], ps)

    # v (token-major): half 0 first so heads 0..5 complete early.
    nc.vector.memset(v_pair[:, :, :, 64:65], W8)
    for half in range(2):
        rhs_sl = slice(half * 384, (half + 1) * 384)
        for mt in range(mt_n):
            ps = ps_mm.tile([P, 384], F32, tag="mm")
            for c in range(3):
                nc.tensor.matmul(
                    ps, z1T[:, 2 * c:2 * c + 2, mt * P:(mt + 1) * P],
                    wv_sb[:, 2 * c:2 * c + 2, rhs_sl],
                    start=(c == 0), stop=(c == 2), perf_mode=DR)
            dst = v_pair[:, mt // 2,
                         slice(12 * half + (mt % 2), 12 * half + 12, 2), 0:64]
            nc.vector.tensor_copy(
                dst, ps.rearrange("p (h d) -> p h d", d=64))

    if upto <= 2:
        nc.sync.dma_start(dbg["z1"][:, :, :], z1T)
        nc.sync.dma_start(dbg["k"][:, :, :], kT)
        nc.sync.dma_start(dbg["q"][:, :, :], qT)
        nc.sync.dma_start(dbg["v"][:, :, :, :], v_pair)
        _rel(p_z1)
        _rel(p_wqk)
        _cut()
        return
    _rel(p_z1)
    _rel(p_wqk)

    # ---- attention/MLP pools (reuse the phase-1/2 SBUF) --------------------
    p_pT = _pool(name="p_pT", bufs=3)
    p_attn = _pool(name="p_attn", bufs=2)
    p_x1 = _pool(name="p_x1", bufs=1)
    p_z2 = _pool(name="p_z2", bufs=1)
    p_h = _pool(name="p_h", bufs=1)
    p_w12 = _pool(name="p_w12", bufs=2)
    p_pw = _pool(name="p_pw", bufs=1)
    p_res = _pool(name="p_res", bufs=1)
    p_st2 = _pool(name="p_st2", bufs=2)
    p_x2 = _pool(name="p_x2", bufs=2)

    pw_sb = p_pw.tile([P, dc, D], BF16)
    nc.sync.dma_start(pw_sb, io["pwT"][:, :, :])
    x1T = p_x1.tile([P, dc, no], BF16)

    def attention(g, attnT, pairs):
        sl = slice(g * 512, (g + 1) * 512)
        for hp in pairs:
            pTs = [p_pT.tile([P, mt_n, 512], FP8, tag="pT",
                             name=f"pT_{g}_{2 * hp + s}") for s in range(2)]
            for mtp in range(mtp_n):
                scs = [ps_sc.tile([P, 2, 512], F32, tag="sc",
                                  name=f"sc_{g}_{hp}_{mtp}_{s}")
                       for s in range(2)]
                for s2 in range(2):
                    mt = 2 * mtp + s2
                    msl = slice(mt * P, (mt + 1) * P)
                    # two heads run concurrently in the two row-halves
                    nc.tensor.matmul(scs[0][:, s2], kT[0:HD, hp, msl],
                                     qT[0:HD, hp, sl])
                    nc.tensor.matmul(scs[1][:, s2], kT[HD:P, hp, msl],
                                     qT[HD:P, hp, sl])
                for s in range(2):
                    nc.scalar.activation(pTs[s][:, 2 * mtp:2 * mtp + 2],
                                         scs[s][:, :, :], AF.Exp,
                                         bias=expb_sb[:, 0:1],
                                         scale=EXP_SCALE)
            if dbg and upto <= 3 and g == 0 and hp == 0:
                nc.sync.dma_start(dbg["pT"][:, :, :], pTs[0])
            for s in range(2):
                h = 2 * hp + s
                po = ps_po.tile([P, 512], F32, tag="po")
                for mtp in range(mtp_n):
                    nc.tensor.matmul(po[0:65],
                                     v_pair[:, mtp, 2 * h:2 * h + 2, 0:65],
                                     pTs[s][:, 2 * mtp:2 * mtp + 2, :],
                                     start=(mtp == 0), stop=(mtp == mtp_n - 1),
                                     perf_mode=DR)
                den = p_rec.tile([1, 512], F32, tag="den", bufs=1)
                nc.vector.tensor_copy(den, po[64:65, :])
                rec = p_rec.tile([1, 512], F32, tag="rec", bufs=1)
                nc.vector.reciprocal_approx_fast(out=rec, in_=den)
                rbc = p_rec.tile([HD, 512], F32, tag="rbc", bufs=2)
                nc.gpsimd.partition_broadcast(rbc, rec, channels=HD)
                if dbg and upto <= 3 and g == 0 and h == 0:
                    po_d = tmps.tile([P, 512], F32, tag="po_d", bufs=1)
                    nc.vector.tensor_copy(po_d, po)
                    nc.sync.dma_start(dbg["po"][:, :], po_d)
                    nc.sync.dma_start(dbg["rec"][:, :], rec)
                    nc.sync.dma_start(dbg["rbc"][:, :], rbc)
                nc.vector.tensor_tensor(attnT[s * HD:(s + 1) * HD, hp, :],
                                        po[0:HD, :], rbc, OP.mult)

    def proj_ln2(g, attnT):
        sl = slice(g * 512, (g + 1) * 512)
        res_sb = p_res.tile([P, dc, 512], BF16, tag="res", bufs=1,
                    name=f"res_{g}")
        for c in range(dc):
            nc.sync.dma_start(res_sb[:, c], io["resid"][:, c, sl])
        for ec in range(dc):
            ps = ps_mm.tile([P, 512], F32, tag="mm")
            for c in range(dc):
                nc.tensor.matmul(ps, pw_sb[:, c, ec * P:(ec + 1) * P],
                                 attnT[:, c, :], start=(c == 0),
                                 stop=(c == dc - 1))
            nc.vector.tensor_add(x1T[:, ec, sl], ps, res_sb[:, ec])
        # LN2 on this 512-token group
        nm2 = p_st2.tile([P, 512], BF16, tag="nm2", bufs=1,
                 name=f"nm2_{g}")
        rstd2 = p_st2.tile([P, 512], BF16, tag="rstd2", bufs=1,
                   name=f"rstd2_{g}")
        var2 = p_st2.tile([P, 512], F32, tag="var2", bufs=1,
                  name=f"var2_{g}")
        _ln_stats(nc, ps_mm, tmps, ones_bf, x1T, sl, nm2, var2, slice(0, 512))
        _ln_rstd(nc, p_st2, eps_sb, var2, rstd2, 512, nm=f"{g}_")
        z2T = p_z2.tile([P, dc, 512], BF16, tag="z2", name=f"z2_{g}")
        for c in range(dc):
            t = tmps.tile([P, 512], BF16, tag="lnt", bufs=3)
            nc.vector.tensor_add(t, x1T[:, c, sl], nm2)
            nc.vector.tensor_mul(z2T[:, c], t, rstd2)
        return z2T

    def fc1_slices(g, z2T, hT, slices, exp_gelu=False):
        for i in slices:
            w1s = p_w12.tile([P, dc, 512], BF16, tag="w1", name=f"w1_{g}_{i}")
            nc.sync.dma_start(w1s, io["w1T"][i])
            for ci in range(4):
                cc = i * 4 + ci
                ps = ps_mm.tile([P, 512], F32, tag="mm")
                for c in range(dc):
                    nc.tensor.matmul(ps, w1s[:, c, ci * P:(ci + 1) * P],
                                     z2T[:, c], start=(c == 0),
                                     stop=(c == dc - 1))
                if exp_gelu:
                    # gelu(x) ~= x*sigmoid(1.702x) using the exp table set
                    # (no ACT table switch inside the attention exp window;
                    # requires zero fc1 bias).
                    e = tmps.tile([P, 512], F32, tag="ge", bufs=2,
                                  name=f"ge_{g}_{cc}")
                    nc.scalar.activation(e, ps, AF.Exp,
                                         bias=zero_sb[:, 0:1], scale=-1.702)
                    nc.vector.tensor_scalar(e, e, one_sb[:, 0:1],
                                            None, OP.add)
                    r = tmps.tile([P, 512], F32, tag="gr", bufs=2,
                                  name=f"gr_{g}_{cc}")
                    nc.vector.reciprocal_approx_fast(out=r, in_=e)
                    nc.vector.tensor_tensor(hT[:, cc], ps, r, OP.mult)
                else:
                    nc.scalar.activation(hT[:, cc], ps, AF.Gelu,
                                         bias=b1p_sb[:, cc:cc + 1])

    def fc2(g, hT):
        sl = slice(g * 512, (g + 1) * 512)
        for ec in range(dc):
            w2s = p_w12.tile([P, hc, P], BF16, tag="w2", name=f"w2_{g}_{ec}")
            nc.sync.dma_start(w2s, io["w2T"][ec])
            ps = ps_mm.tile([P, 512], F32, tag="mm")
            for c in range(hc):
                nc.tensor.matmul(ps, w2s[:, c], hT[:, c],
                                 start=(c == 0), stop=(c == hc - 1))
            x2 = p_x2.tile([P, 512], F32, tag="x2")
            if with_fc2_bias:
                t = tmps.tile([P, 512], F32, tag="f2t")
                nc.vector.tensor_scalar(t, ps, fc2b_sb[:, ec:ec + 1], None,
                                        OP.add)
                nc.vector.tensor_add(x2, t, x1T[:, ec, sl])
            else:
                nc.vector.tensor_add(x2, ps, x1T[:, ec, sl])
            nc.sync.dma_start(io["outT"][:, ec, sl], x2)

    # pipeline: attention(g1) emission is interleaved with fc1(g0) in
    # chunks so the exp-bound attention window keeps the tensor engine fed
    # (and the HAM clock-gate warm) with MLP matmuls; fc2(g0) follows to
    # fill the tail of the exp window.
    attnT0 = p_attn.tile([P, dc, 512], BF16, tag="attnT", name="attnT_0")
    attention(0, attnT0, range(dc))
    if upto <= 3:
        nc.sync.dma_start(dbg["attn"][:, :, :], attnT0)
        _cut()
        return
    z2_0 = proj_ln2(0, attnT0)
    if upto <= 4:
        nc.sync.dma_start(dbg["x1"][:, :, :], x1T[:, :, 0:512])
        nc.sync.dma_start(dbg["attn"][:, :, :], attnT0)
        _cut()
        return
    attnT1 = p_attn.tile([P, dc, 512], BF16, tag="attnT", name="attnT_1")
    hT0 = p_h.tile([P, hc, 512], BF16, tag="hT", name="hT_0")
    for chunk in range(3):
        attention(1, attnT1, [2 * chunk, 2 * chunk + 1])
        fc1_slices(0, z2_0, hT0, [2 * chunk, 2 * chunk + 1],
                   exp_gelu=not with_qk_bias)
    fc2(0, hT0)
    z2_1 = proj_ln2(1, attnT1)
    hT1 = p_h.tile([P, hc, 512], BF16, tag="hT", name="hT_1")
    fc1_slices(1, z2_1, hT1, range(6))
    fc2(1, hT1)
    _cut()


# --------------------------------------------------------------------------
# Host side
# --------------------------------------------------------------------------

_NC_CACHE = {}


def _get_nc(nt, no, with_qk_bias, with_fc2_bias, reps=1, upto=99):
    key = (nt, no, with_qk_bias, with_fc2_bias, reps, upto)
    if key not in _NC_CACHE:
        _NC_CACHE[key] = _build_nc(nt, no, with_qk_bias, with_fc2_bias, reps,
                                   upto)
    return _NC_CACHE[key]


def _prep_weights(ln1_w, ln1_b, qkv_w, qkv_b, proj_w, proj_b,
                  ln2_w, ln2_b, fc1_w, fc1_b, fc2_w, fc2_b):
    w_qkv = qkv_w * ln1_w[None, :]
    b_qkv = qkv_w @ ln1_b + qkv_b
    pb = proj_b + proj_w @ b_qkv[2 * D:]
    w1 = fc1_w * ln2_w[None, :]
    b1p = fc1_b + fc1_w @ ln2_b

    def col(v, chunks):
        return np.ascontiguousarray(v.reshape(chunks, P).T.astype(np.float32))

    def sb(wT, chunks, npdt):
        # [K, M] -> [P, chunks, M] with K = chunks*P (SBUF layout)
        k, m = wT.shape
        return np.ascontiguousarray(
            wT.reshape(chunks, P, m).transpose(1, 0, 2).astype(npdt))

    w1_s = sb(w1.T, DC, NPBF16)                   # [P, DC, 3072]
    w2_s = sb(fc2_w.T, HC, NPBF16)                # [P, HC, 768]
    shared = {
        "wqkT": sb(w_qkv[:2 * D].T * W8, DC, NPFP8),
        "wvT": sb(w_qkv[2 * D:].T * W8, DC, NPFP8),
        "pwT": sb(proj_w.T, DC, NPBF16),
        "w1T": np.ascontiguousarray(
            w1_s.reshape(P, DC, 6, 512).transpose(2, 0, 1, 3)),
        "w2T": np.ascontiguousarray(
            w2_s.reshape(P, HC, 6, P).transpose(2, 0, 1, 3)),
        "qk_bias": col(b_qkv[:2 * D] * W8, 2 * DC),
        "b1p": col(b1p, HC),
        "fc2_b": col(fc2_b, DC),
    }
    flags = (bool(np.any(b_qkv[:2 * D])) or bool(np.any(b1p)),
             bool(np.any(fc2_b)))
    return shared, pb, flags


def run_on_device(inputs, trace=False):
    x = np.asarray(inputs["x"], dtype=np.float32)
    nb, nt, d = x.shape
    no = nt // 2
    args = {k: np.asarray(v, dtype=np.float32) for k, v in inputs.items()
            if k != "x"}
    shared, pb, (f_qk, f_f2) = _prep_weights(
        args["ln1_w"], args["ln1_b"], args["qkv_w"], args["qkv_b"],
        args["proj_w"], args["proj_b"], args["ln2_w"], args["ln2_b"],
        args["fc1_w"], args["fc1_b"], args["fc2_w"], args["fc2_b"])

    nc = _get_nc(nt, no, f_qk, f_f2)

    in_maps = []
    for core in range(N_CORES):
        b, g = divmod(core, 2)
        xr = np.roll(x[b], -g * no, axis=0)
        m = dict(shared)
        m["xT"] = np.ascontiguousarray(
            xr.T.reshape(DC, P, nt).transpose(1, 0, 2)).astype(NPBF16)
        rs = x[b, g * no:(g + 1) * no].T + pb[:, None]
        m["resid"] = np.ascontiguousarray(
            rs.reshape(DC, P, no).transpose(1, 0, 2)).astype(NPBF16)
        in_maps.append(m)

    res = run_bass_kernel_spmd(nc, in_maps, core_ids=list(range(N_CORES)),
                               trace=trace)
    out = np.empty((nb, nt, d), dtype=np.float32)
    for core in range(N_CORES):
        b, g = divmod(core, 2)
        o = res.results[core]["outT"]          # [P, DC, no]
        out[b, g * no:(g + 1) * no, :] = o.transpose(1, 0, 2).reshape(d, no).T
    return out, res


def kernel(**inputs) -> np.ndarray:
    out, _ = run_on_device(inputs, trace=False)
    return out
